# revision 12
# baseline (speedup 1.0000x reference)
"""Trainium2 Bass kernel for nn_AlgebraicAttention (8-core SPMD).

Sharding: core c -> batch b = c//4, head quartet column g = c%4.  Heads are
sorted by ALiBi reach d_h = 17/slope_h (descending) and grouped into four
rank-quartets; program head-slot s on core g runs head quartets[s][g].  Every
core executes the identical program with identical tile geometry (required:
one SPMD program for all 8 cores); per-core data (weights, tail constants)
carries the head differences.  Each core computes its 4 heads' attention and
a partial out-projection; the host sums the 8 partials.

Math notes:
  - scores^T layout [j (keys, partitions), i (queries, free)].
  - ALiBi (j-i)*slope folded into the QK^T contraction via 4 extra f16 rows
    (hi/lo splits of -i*slope and j*slope).
  - geometric cut: per slot, only (i,j) pairs with dist = i-j <= Dm[slot]
    (Dm = ceil(17/min_slope_of_quartet)) are computed.  Off-diag tiles
    narrow to we columns (64-rounded); fully-far tiles drop entirely; diag
    windows narrow to wed and the per-slot 0.5-valued tri mask also zeroes
    pairs with dist > Dm.  The dropped pairs' contribution to the softmax
    DENOMINATOR is systematic (all-positive); it is precomputed on the host
    as tail(h, i) = sum_dropped num(alibi) and added back inside the
    reciprocal DVE op (RECIPT).  The dropped numerator contribution is a
    random-sign sum of O(1e-3) weights -> statistically negligible.
  - rational softmax numerator num = 0.5*(1 + x/sqrt(1+x^2)) computed as
    sin(arctan(x)) in 2 table-based ScalarE passes + a cheap DVE f16 affine
    (off-diag) or tri-mask multiply (diag).
  - the +0.5*mask constant part of diag num comes from triangular matmuls
    in P@V (also supplying the denominator's diag mass); the denominator
    itself is a ones-column in the P@V matmul, inverted with the 1-Newton
    RECIPT op (max rel err ~0.17%) that also adds the tail constant.
"""

import numpy as np

import concourse.bass as bass
import concourse.mybir as mybir
from concourse import bacc
from concourse.tile import TileContext
from concourse.bass_utils import run_bass_kernel_spmd

# --------------------------------------------------------------------------
# Custom DVE op: out = approx 1/(Src0 + Src1) (1-Newton, ~0.17% max rel err)
# --------------------------------------------------------------------------
import concourse.dve_ops as dve_ops
from concourse.dve_ops import DveOp
from concourse.dve_spec import (
    AluOp, Bin, C0, C1, C2, C3, Spec, Src0, Src1, _spill_c3_to_src1, lower, sq,
)
from concourse.dve_uop import DveOpSpec

RC0 = -0.23548383
RC1 = 2.00161239
RC2 = 1.00011986
AB0 = RC0 * float(np.sqrt(RC2))
AB1 = RC1 * float(np.sqrt(RC2))

# deg-5 odd minimax sin on arctan range (max num abs err ~3e-5)
S5C0 = 0.9997329
S5C1 = -0.16575311
S5C2 = 0.00754758


def _notf(a):
    return (~np.asarray(a, np.float32).view(np.int32)).view(np.float32)


def _ref_recipt(in0, in1, c0, c1, c2):
    s = np.asarray(in0, np.float32) + np.asarray(in1, np.float32)
    y0 = _notf(s) * np.float32(c0)
    return (y0 * (np.float32(c1) - s * y0)).astype(np.float32)


def _spec_recipt():
    s = Bin(AluOp.ADD, Src0, Src1)
    n = Bin(AluOp.BITWISE_NOT, s, s)
    y0 = n * C0
    y1 = y0 * (C1 - s * y0)
    return Spec(body=y1, reference=_ref_recipt)


def _ref_sinm(in0, in1, c0, c1, c2):
    # masked sin(theta): (theta*mask) * P(theta^2); mask carries the 0.5
    th = np.asarray(in0, np.float32)
    m = np.asarray(in1, np.float32)
    u = th * th
    p = np.float32(c0) + u * (np.float32(c1) + u * np.float32(c2))
    return ((th * m) * p).astype(np.float32)


def _spec_sinm():
    m0 = Src0 * Src1
    u = sq(Src0)
    p = C0 + u * (C1 + u * C2)
    return Spec(body=m0 * p, reference=_ref_sinm)


def _ref_sina(in0, in1, c0, c1, c2):
    # 0.5 + theta*P(theta^2) with 0.5-scaled coeffs; in1 = [P,1] 0.5 const
    th = np.asarray(in0, np.float32)
    u = th * th
    p = np.float32(c0) + u * (np.float32(c1) + u * np.float32(c2))
    return (np.asarray(in1, np.float32) + th * p).astype(np.float32)


def _spec_sina():
    u = sq(Src0)
    p = C0 + u * (C1 + u * C2)
    return Spec(body=_spill_c3_to_src1(C3 + Src0 * p), reference=_ref_sina)


def _register(name, spec, subdim=False):
    for op in dve_ops.OPS:
        if op.name == name:
            return op
    opcode = dve_ops._CUSTOM_DVE_ROW_BASE + len(dve_ops.OPS)
    assert opcode < 0x20
    rd1_en = dve_ops.has_src1(spec)
    shas = {}
    for ver in ("v3", "v4"):
        try:
            uops = lower(spec, ver=ver)
            shas[ver] = DveOpSpec(name=name, opcode=opcode, uops=uops,
                                  rd1_en=rd1_en).sha(ver)
        except Exception:
            pass
    op = DveOp(name, spec, subdim, uops_sha=shas)
    dve_ops.OPS.append(op)
    dve_ops._SUB_OPCODE_FOR_NAME[name] = opcode
    dve_ops.CUSTOM_DVE_SPECS[name] = spec
    return op


RECIPT_ANT = _register("RECIPT_ANT", _spec_recipt())
SINM_ANT = _register("SINM_ANT", _spec_sinm())
SINA_ANT = _register("SINA_ANT", _spec_sina())

# diag-tile geometry: for dd = jt-4*tau in 0..3 the i-window of the
# [128 j, 512 i] o_ps block is [IL[dd], IL[dd]+WD[dd]); within it
# dist = c - p (window col c, partition p).
IL = [0, 128, 256, 384]
WD = [512, 384, 256, 128]

# --------------------------------------------------------------------------
# Problem constants
# --------------------------------------------------------------------------
B, T, C, H, D = 2, 2048, 1024, 16, 64
NCORES = 8
HPC = 4                 # heads per core
SCALE = 1.0 / 8.0       # 1/sqrt(D)
DEXT = D + 4            # q/k + [islope_hi, islope_lo, 1, 1] / [1, 1, jhi, jlo]
NT = T // 512           # 4 i-chunks of 512
NJT = T // 128          # 16 j-tiles of 128
DFAR = 17.0             # |alibi| beyond which num < ~1e-3 (cut distance)

F32 = mybir.dt.float32
F16 = mybir.dt.float16
AF = mybir.ActivationFunctionType

_PROG = {}


def _ceil64(x):
    return int(np.ceil(x / 64.0)) * 64


def _plan(slopes=None):
    """Head->slot assignment and per-slot computed-width tables."""
    if slopes is None:
        start = 2.0 ** (-8.0 / H)
        slopes = np.asarray([start ** (i + 1) for i in range(H)], np.float32)
    slopes = np.asarray(slopes, np.float32)
    d = DFAR / np.maximum(np.abs(slopes), 1e-12)
    order = np.argsort(-d, kind="stable")
    quartets = [order[4 * s:4 * s + 4].tolist() for s in range(4)]
    Dm = [int(np.ceil(max(float(d[h]) for h in quartets[s])))
          for s in range(4)]
    weoff = {}
    for s in range(4):
        for tau in range(NT):
            for jt in range(4 * tau):
                we = 128 * jt + 128 + Dm[s] - 512 * tau
                weoff[(s, tau, jt)] = min(max(_ceil64(we), 0), 512)
    wed = [[min(WD[dd], _ceil64(128 + Dm[s])) for dd in range(4)]
           for s in range(4)]
    key = tuple(Dm)
    return quartets, Dm, weoff, wed, key


# --------------------------------------------------------------------------
# Device program (identical on all 8 cores)
# --------------------------------------------------------------------------
def _build_program(reps=1, slopes=None):
    import os
    dbg = os.environ.get("BASSDBG", "")
    _, Dm, weoff, wed, key = _plan(slopes)
    cache_key = (reps, key, dbg)
    if cache_key in _PROG:
        return _PROG[cache_key]

    nc = bacc.Bacc("TRN2", target_bir_lowering=False, debug=False,
                   num_devices=NCORES)

    d_xT = nc.dram_tensor("xT", [NT, 2, 128, 4, 512], F16,
                          kind="ExternalInput")
    d_wqk = nc.dram_tensor("wqk", [128, 8, 512], F16, kind="ExternalInput")
    d_wv = nc.dram_tensor("wv", [128, 8, 256], F16, kind="ExternalInput")
    d_wo = nc.dram_tensor("wo", [128, 2, 1024], F16, kind="ExternalInput")
    d_qext = nc.dram_tensor("qext", [4, 4, T], F16, kind="ExternalInput")
    d_kext = nc.dram_tensor("kext", [4, 4, T], F16, kind="ExternalInput")
    d_tri = nc.dram_tensor("trimask", [128, 4, 512], F16,
                           kind="ExternalInput")
    d_tail = nc.dram_tensor("tail", [64, 4, T], F16, kind="ExternalInput")
    d_out = nc.dram_tensor("out_p", [T, C], F16, kind="ExternalOutput")

    with TileContext(nc) as tc:
        with (
            tc.tile_pool(name="const", bufs=1) as cpool,
            tc.tile_pool(name="ew", bufs=6) as ew,
            tc.tile_pool(name="osb", bufs=3) as osb,
            tc.tile_pool(name="acc", bufs=2, space="PSUM") as accp,
            tc.tile_pool(name="ps", bufs=4, space="PSUM") as psp,
            tc.tile_pool(name="pso", bufs=2, space="PSUM") as psop,
        ):
            # ---------------- persistent tensors ----------------
            wqk_sb = cpool.tile([128, 8, 512], F16, tag="wqk")
            wv_sb = cpool.tile([128, 8, 256], F16, tag="wv")
            wo_sb = cpool.tile([128, 2, 1024], F16, tag="wo")
            q_all = cpool.tile([128, HPC, T], F16, tag="q_all")
            k_all = cpool.tile([128, HPC, T], F16, tag="k_all")
            v_sb = cpool.tile([128, NJT, HPC * 128], F16, tag="v_sb")
            o_all = cpool.tile([128, 2, T], F16, tag="o_all")
            tri = cpool.tile([128, 4, 512], F16, tag="tri")
            tail_sb = cpool.tile([64, 4, T], F16, tag="tail")
            xsb = cpool.tile([128, NT, 2, 4, 512], F16, tag="xsb")

            nc.sync.dma_start(wqk_sb[:], d_wqk[:])
            nc.sync.dma_start(wv_sb[:], d_wv[:])
            nc.sync.dma_start(wo_sb[:], d_wo[:])
            nc.sync.dma_start(tri[:], d_tri[:])
            nc.sync.dma_start(tail_sb[:], d_tail[:])
            for tau in range(NT):
                for half in range(2):
                    nc.sync.dma_start(xsb[:, tau, half], d_xT[tau, half])

            # constants: hoisted out of the timing rep-loop (idempotent).
            # ext rows:   even slots at rows [64:68) (matmul reads [0:68)),
            # odd slots at rows [60:64) with zeros in [0:60) (matmul reads
            # [0:128) — ldweights requires partition base 0 for >32 rows).
            for h in range(HPC):
                if h % 2 == 1:
                    nc.vector.memset(q_all[0:64, h, :], 0.0)
                    nc.vector.memset(k_all[0:64, h, :], 0.0)
                base = 64 if h % 2 == 0 else 60
                nc.sync.dma_start(q_all[base:base + 4, h, :], d_qext[:, h, :])
                nc.sync.dma_start(k_all[base:base + 4, h, :], d_kext[:, h, :])

            half = cpool.tile([128, 1], F32, tag="half")
            nc.vector.memset(half[:], 0.5)
            if dbg == "noew":
                num_const = cpool.tile([128, 512], F16, tag="numc")
                nc.vector.memset(num_const[:], 0.001)
            if dbg in ("nopv", "proj_only", "nodma", "p1only"):
                nc.vector.memset(o_all[:], 0.001)
            # ones columns of V_ext in cols 0:64 of each head's group, so
            # the P@V denominator lands at o_ps partitions [0:64) (custom DVE
            # ops require partition base 0 on their input).
            v4 = v_sb[:].rearrange("p t (h e) -> p t h e", e=128)
            nc.gpsimd.memset(v4[:, :, :, 0:64], 1.0)

            import contextlib
            loop_ctx = (tc.For_i(0, reps, 1) if reps > 1
                        else contextlib.nullcontext())
            with loop_ctx:
              # Issue order per round tau:
              #   pairA(tau) -> phase3(tau-1) -> pairB(tau) -> phase1(tau+1)
              # The PE-only projection segments are sandwiched between
              # attention pairs, so the elementwise engines drain their
              # attention backlog while the PE runs projections, instead of
              # idling per round (phase1(0) is the prologue, phase3(NT-1)
              # the epilogue).
              LA = 5  # QK lookahead within a pair (psp ring bounds it too)

              # build-time per-engine load model (ns) for assigning each
              # tile's sin+finisher to ScalarE/DVE/Pool (greedy min-max).
              LOAD = {"sc": 0.0, "dve": 0.0, "pool": 0.0}

              def phase1(tau):
                  ts = slice(512 * tau, 512 * tau + 512)
                  xa = xsb[:, tau, 0]
                  xb = xsb[:, tau, 1]

                  # V projection first: next round's diag/tri matmuls need it
                  for ttl in range(4):
                      tt = 4 * tau + ttl
                      accv = accp.tile([128, 256], F32, tag="acc",
                                       name="accv")
                      for kt in range(8):
                          xt = xa if kt < 4 else xb
                          nc.tensor.matmul(
                              accv[:], xt[:, kt % 4, 128 * ttl:128 * ttl + 128],
                              wv_sb[:, kt, :],
                              start=(kt == 0), stop=(kt == 7))
                      nc.vector.tensor_copy(
                          out=v4[:, tt, :, 64:128],
                          in_=accv[:].rearrange("p (h e) -> p h e", e=64))
                      LOAD["dve"] += 327

                  for mt in (0, 2, 1, 3):
                      acc = accp.tile([128, 512], F32, tag="acc", name="acc")
                      for kt in range(8):
                          xt = xa if kt < 4 else xb
                          nc.tensor.matmul(
                              acc[:], wqk_sb[:, kt, 128 * mt:128 * mt + 128],
                              xt[:, kt % 4, :],
                              start=(kt == 0), stop=(kt == 7))
                      dst = q_all if mt < 2 else k_all
                      h0 = 2 * (mt % 2)
                      nc.vector.tensor_copy(out=dst[0:64, h0, ts],
                                            in_=acc[0:64, :])
                      nc.vector.tensor_copy(out=dst[64:128, h0 + 1, ts],
                                            in_=acc[64:128, :])
                      LOAD["dve"] += 2 * 593

              def phase3(tau):
                  for ttl in range(4):
                      tt = 4 * tau + ttl
                      for oc in range(2):
                          acc = accp.tile([128, 512], F32, tag="acc",
                                          name="acc3")
                          for half in range(2):
                              nc.tensor.matmul(
                                  acc[:],
                                  o_all[:, half, 128 * tt:128 * tt + 128],
                                  wo_sb[:, half, 512 * oc:512 * oc + 512],
                                  start=(half == 0), stop=(half == 1))
                          ot = osb.tile([128, 512], F16, tag="ot", name="ot")
                          nc.vector.tensor_copy(out=ot[:], in_=acc[:])
                          LOAD["dve"] += 594
                          if dbg != "nodma":
                              nc.sync.dma_start(
                                  d_out[128 * tt:128 * tt + 128,
                                        512 * oc:512 * oc + 512],
                                  ot[:])

              def attn_pair(tau, hA, hB):
                  i0 = 512 * tau
                  isl = slice(i0, i0 + 512)
                  njt = 4 * (tau + 1)
                  blocks = []
                  for h in (hA, hB):
                      blocks.append(dict(
                          h=h,
                          hb=0, hk=(DEXT if h % 2 == 0 else 128),
                          o_ps=psop.tile([128, 512], F32, tag="pso",
                                         name="o_ps")))

                  def geom(h, n):
                      dd = n - 4 * tau
                      if dd >= 0:
                          return IL[dd], wed[h][dd]
                      return 0, weoff[(h, tau, n)]

                  def qk(bi, n):
                      blk = blocks[bi]
                      il, wd = geom(blk["h"], n)
                      x_ps = psp.tile([128, 512], F32, tag="ps", name="x_ps")
                      nc.tensor.matmul(
                          x_ps[:, 0:wd],
                          k_all[blk["hb"]:blk["hb"] + blk["hk"],
                                blk["h"], 128 * n:128 * n + 128],
                          q_all[blk["hb"]:blk["hb"] + blk["hk"],
                                blk["h"], i0 + il:i0 + il + wd],
                          start=True, stop=True)
                      return x_ps

                  # wide tiles first, narrow tiles last: the round's tail is
                  # then short elementwise chains, minimizing the in-order PE
                  # bubble at the pair boundary.
                  sched = [(bi, n) for n in range(njt) for bi in (0, 1)
                           if geom(blocks[bi]["h"], n)[1] > 0]
                  sched.sort(key=lambda s: -geom(blocks[s[0]]["h"], s[1])[1])
                  total = {0: 0, 1: 0}
                  for bi, n in sched:
                      total[bi] += 1

                  tiles = {}
                  for idx in range(min(LA, len(sched))):
                      tiles[sched[idx]] = qk(*sched[idx])
                  # group openers: constant +0.5*mask part of the diag tiles
                  # (no elementwise dependency -> PE never waits)
                  for bi in (0, 1):
                      h = blocks[bi]["h"]
                      for dd in range(4):
                          jt = 4 * tau + dd
                          nc.tensor.matmul(
                              blocks[bi]["o_ps"][:, IL[dd]:IL[dd] + WD[dd]],
                              v_sb[:, jt, 128 * h:128 * h + 128],
                              tri[:, h, 0:WD[dd]],
                              start=(dd == 0), stop=False,
                              skip_group_check=True)
                  done = {0: 0, 1: 0}
                  for idx, (bi, jt) in enumerate(sched):
                      if idx + LA < len(sched):
                          tiles[sched[idx + LA]] = qk(*sched[idx + LA])
                      x_ps = tiles.pop((bi, jt))
                      blk = blocks[bi]
                      h = blk["h"]
                      dd = jt - 4 * tau  # >= 0 on diagonal block
                      il, wd = geom(h, jt)
                      if dbg == "noew":
                          num = num_const
                      else:
                          num = ew.tile([128, 512], F16, tag="num",
                                        name="num")
                          # x/sqrt(1+x^2) = sin(arctan(x)): ScalarE arctan,
                          # then either a ScalarE Sin pass + DVE/Pool f16
                          # finisher (affine / tri-mask mul), or a single
                          # fused deg-5 sin-poly DVE op with the finisher
                          # folded in.  Greedy min-max over the modeled
                          # engine loads picks per tile.  x_ps is freed
                          # right after the arctan pass.
                          at = ew.tile([128, 512], F32, tag="at", name="at")
                          nc.scalar.activation(at[:, 0:wd], x_ps[:, 0:wd],
                                               AF.Arctan)
                          LOAD["sc"] += wd * 1.043 + 60
                          c_sin = wd * 1.043 + 60
                          c_fin_dve = (wd * 0.52 + 60) if dd >= 0 else (
                              wd * 0.30 + 60)
                          c_fin_pool = wd * 2.48 + 95
                          c_fused = wd * 1.043 + 125
                          mA = max(LOAD["sc"] + c_sin,
                                   LOAD["dve"] + c_fin_dve, LOAD["pool"])
                          mB = max(LOAD["sc"] + c_sin, LOAD["dve"],
                                   LOAD["pool"] + c_fin_pool)
                          mC = max(LOAD["sc"], LOAD["dve"] + c_fused,
                                   LOAD["pool"])
                          if mC <= mA and mC <= mB:
                              # fused DVE sin (+mask / +0.5 affine)
                              LOAD["dve"] += c_fused
                              if dd >= 0:
                                  nc.vector._custom_dve(
                                      SINM_ANT, out=num[:, 0:wd],
                                      in0=at[:, 0:wd], in1=tri[:, h, 0:wd],
                                      s0=S5C0, s1=S5C1, imm2=S5C2)
                              else:
                                  nc.vector._custom_dve(
                                      SINA_ANT, out=num[:, 0:wd],
                                      in0=at[:, 0:wd], in1=half[:],
                                      s0=0.5 * S5C0, s1=0.5 * S5C1,
                                      imm2=0.5 * S5C2)
                          else:
                              un = ew.tile([128, 512], F16, tag="un",
                                           name="un")
                              nc.scalar.activation(un[:, 0:wd], at[:, 0:wd],
                                                   AF.Sin)
                              LOAD["sc"] += c_sin
                              eng = nc.vector if mA <= mB else nc.gpsimd
                              LOAD["dve" if mA <= mB else "pool"] += (
                                  c_fin_dve if mA <= mB else c_fin_pool)
                              if dd < 0:
                                  eng.tensor_scalar(
                                      out=num[:, 0:wd], in0=un[:, 0:wd],
                                      scalar1=0.5, scalar2=0.5,
                                      op0=mybir.AluOpType.mult,
                                      op1=mybir.AluOpType.add)
                              else:
                                  eng.tensor_mul(
                                      out=num[:, 0:wd], in0=un[:, 0:wd],
                                      in1=tri[:, h, 0:wd])
                      done[bi] += 1
                      if dbg != "nopv":
                          nc.tensor.matmul(
                              blk["o_ps"][:, il:il + wd],
                              v_sb[:, jt, 128 * h:128 * h + 128],
                              num[:, 0:wd],
                              start=False, stop=(done[bi] == total[bi]),
                              skip_group_check=True)

                  # denominators (rows 0:64, replicated by the ones columns;
                  # custom-DVE ops require partition base 0 on their input).
                  # RECIPT adds the host-precomputed dropped-tail mass and
                  # inverts in one op.
                  for bi in (0, 1) if dbg != "nopv" else ():
                      h = blocks[bi]["h"]
                      o_ps = blocks[bi]["o_ps"]
                      rsb = ew.tile([64, 512], F32, tag="rsb", name="rsb")
                      nc.vector._custom_dve(
                          RECIPT_ANT, out=rsb[:], in0=o_ps[0:64, :],
                          in1=tail_sb[:, h, isl], s0=AB0, s1=AB1)
                      nc.vector.tensor_mul(
                          out=o_all[64 * (h % 2):64 * (h % 2) + 64,
                                    h // 2, isl],
                          in0=o_ps[64:128, :], in1=rsb[:])
                      LOAD["dve"] += 2 * 593

              phase1(0)
              for tau in range(NT):
                  if dbg not in ("proj_only", "nodma", "p1only"):
                      attn_pair(tau, 0, 3)
                  if tau >= 1 and dbg != "p1only":
                      phase3(tau - 1)
                  if dbg not in ("proj_only", "nodma", "p1only"):
                      attn_pair(tau, 1, 2)
                  if tau + 1 < NT:
                      phase1(tau + 1)
              if dbg != "p1only":
                  phase3(NT - 1)

    nc.compile()
    _PROG[cache_key] = nc
    return nc


# --------------------------------------------------------------------------
# Host-side input preparation
# --------------------------------------------------------------------------
def _split2(v):
    v = v.astype(np.float32)
    p1 = v.astype(np.float16).astype(np.float32)
    p2 = (v - p1).astype(np.float16)
    return p1.astype(np.float16), p2


def _computed_mask_for_slot(Dm, weoff_s, wed_s):
    """[T, T] bool over (i, j): True where the pair is computed on-device."""
    keep = np.zeros((T, T), dtype=bool)
    for tau in range(NT):
        i0 = 512 * tau
        for jt in range(4 * tau):
            we = weoff_s[(tau, jt)]
            if we > 0:
                keep[i0:i0 + we, 128 * jt:128 * jt + 128] = True
        for dd in range(4):
            we_d = wed_s[dd]
            j0 = i0 + 128 * dd
            c = np.arange(we_d)[:, None]
            p = np.arange(128)[None, :]
            m = (c - p >= 0) & (c - p <= Dm)
            keep[i0 + IL[dd]:i0 + IL[dd] + we_d, j0:j0 + 128] = m
    return keep


def _host_prep(x, w_qkv, w_out, alibi_slopes):
    x = np.asarray(x, np.float32)
    w_qkv = np.asarray(w_qkv, np.float32)
    w_out = np.asarray(w_out, np.float32)
    slopes = np.asarray(alibi_slopes, np.float32)
    quartets, Dm, weoff, wed, _ = _plan(slopes)

    iarr = np.arange(T, dtype=np.float32)
    # per-slot 0.5-valued tri masks: 0.5 iff 0 <= c - p <= Dm[s]
    p = np.arange(128)[:, None]
    c = np.arange(512)[None, :]
    trimask = np.zeros((128, 4, 512), np.float16)
    for s in range(4):
        trimask[:, s, :] = (((c - p) >= 0) & ((c - p) <= Dm[s])).astype(
            np.float16) * np.float16(0.5)

    # dropped-pair masks + per-(slot-geometry) distance weights are shared
    # across cores; the tail itself depends on the head's slope.
    rel = np.arange(T)[:, None] - np.arange(T)[None, :]
    causal_valid = rel >= 0
    dropped_s = []
    for s in range(4):
        weoff_s = {(tau, jt): weoff[(s, tau, jt)]
                   for tau in range(NT) for jt in range(4 * tau)}
        keep = _computed_mask_for_slot(Dm[s], weoff_s, wed[s])
        dropped_s.append((~keep) & causal_valid)

    def tail_for(s, slope):
        d = np.abs(rel).astype(np.float32) * np.float32(slope)
        num_a = 0.5 * (1.0 - d / np.sqrt(1.0 + d * d))
        return (num_a * dropped_s[s]).sum(axis=1).astype(np.float32)  # [T]

    in_maps = []
    for cc in range(NCORES):
        b = cc // 4
        g = cc % 4
        heads = [quartets[s][g] for s in range(HPC)]

        # pre-swizzled to the SBUF tile layout: [tau, half, p, k, t]
        xTf = np.ascontiguousarray(x[b].T).astype(np.float16)
        xT = np.ascontiguousarray(
            xTf.reshape(2, 4, 128, 4, 512).transpose(3, 0, 2, 1, 4))

        q_rows = np.concatenate(
            [w_qkv[64 * h:64 * h + 64] for h in heads], axis=0) * SCALE
        k_rows = np.concatenate(
            [w_qkv[C + 64 * h:C + 64 * h + 64] for h in heads], axis=0)
        qk_rows = np.concatenate([q_rows, k_rows], axis=0)  # [512, 1024]
        wqk = np.ascontiguousarray(
            qk_rows.T.reshape(8, 128, 512).transpose(1, 0, 2)).astype(np.float16)

        v_rows = np.concatenate(
            [w_qkv[2 * C + 64 * h:2 * C + 64 * h + 64] for h in heads], axis=0)
        wv = np.ascontiguousarray(
            v_rows.T.reshape(8, 128, 256).transpose(1, 0, 2)).astype(np.float16)

        Wg = np.concatenate(
            [w_out[:, 64 * h:64 * h + 64] for h in heads], axis=1)  # [1024,256]
        wo = np.ascontiguousarray(
            Wg.T.reshape(2, 128, 1024).transpose(1, 0, 2)).astype(np.float16)

        qext = np.zeros((4, HPC, T), np.float16)
        kext = np.zeros((4, HPC, T), np.float16)
        tail = np.zeros((64, HPC, T), np.float16)
        for j, h in enumerate(heads):
            sl = float(slopes[h])
            ihi, ilo = _split2(-iarr * sl)
            jhi, jlo = _split2(iarr * sl)
            qext[0, j] = ihi
            qext[1, j] = ilo
            qext[2, j] = 1.0
            qext[3, j] = 1.0
            kext[0, j] = 1.0
            kext[1, j] = 1.0
            kext[2, j] = jhi
            kext[3, j] = jlo
            tail[:, j, :] = tail_for(j, sl)[None, :].astype(np.float16)

        in_maps.append({
            "xT": xT, "wqk": wqk, "wv": wv, "wo": wo,
            "qext": qext, "kext": kext, "trimask": trimask, "tail": tail,
        })
    return in_maps


def _assemble(partials):
    out = np.zeros((B, T, C), np.float32)
    for c in range(NCORES):
        out[c // 4] += partials[c]
    return out.astype(np.float32)


def kernel(x, w_qkv, w_out, alibi_slopes):
    nc = _build_program(slopes=alibi_slopes)
    in_maps = _host_prep(x, w_qkv, w_out, alibi_slopes)
    res = run_bass_kernel_spmd(nc, in_maps, core_ids=list(range(NCORES)))
    return _assemble([r["out_p"] for r in res.results])


# revision 37
# speedup vs baseline: 1.2469x; 1.2469x over previous
"""Trainium2 Bass kernel for nn_AlgebraicAttention (8-core SPMD).

Sharding: core c -> batch b = c//4, head quartet column g = c%4.  Heads are
sorted by ALiBi reach d_h = 17/slope_h (descending) and grouped into four
rank-quartets; program head-slot s on core g runs head quartets[s][g].  Every
core executes the identical program with identical tile geometry (required:
one SPMD program for all 8 cores); per-core data (weights, tail constants)
carries the head differences.  Each core computes its 4 heads' attention and
a partial out-projection; the host sums the 8 partials.

Math notes:
  - scores^T layout [j (keys, partitions), i (queries, free)].
  - ALiBi (j-i)*slope folded into the QK^T contraction via 4 extra f16 rows
    (hi/lo splits of -i*slope and j*slope).
  - geometric cut: per slot, only (i,j) pairs with dist = i-j <= Dm[slot]
    (Dm = ceil(17/min_slope_of_quartet)) are computed.  Off-diag tiles
    narrow to we columns (64-rounded); fully-far tiles drop entirely; diag
    windows narrow to wed and the per-slot 0.5-valued tri mask also zeroes
    pairs with dist > Dm.  The dropped pairs' contribution to the softmax
    DENOMINATOR is systematic (all-positive); it is precomputed on the host
    as tail(h, i) = sum_dropped num(alibi) and added back inside the
    reciprocal DVE op (RECIPT).  The dropped numerator contribution is a
    random-sign sum of O(1e-3) weights -> statistically negligible.
  - rational softmax numerator num = 0.5*(1 + x/sqrt(1+x^2)) computed as
    sin(arctan(x)) in 2 table-based ScalarE passes + a cheap DVE f16 affine
    (off-diag) or tri-mask multiply (diag).
  - the +0.5*mask constant part of diag num comes from triangular matmuls
    in P@V (also supplying the denominator's diag mass); the denominator
    itself is a ones-column in the P@V matmul, inverted with the 1-Newton
    RECIPT op (max rel err ~0.17%) that also adds the tail constant.
"""

import numpy as np

import concourse.bass as bass
import concourse.mybir as mybir
from concourse import bacc
from concourse.tile import TileContext
from concourse.bass_utils import run_bass_kernel_spmd

# --------------------------------------------------------------------------
# Custom DVE op: out = approx 1/(Src0 + Src1) (1-Newton, ~0.17% max rel err)
# --------------------------------------------------------------------------
import concourse.dve_ops as dve_ops
from concourse.dve_ops import DveOp
from concourse.dve_spec import (
    AluOp, Bin, C0, C1, C2, C3, Spec, Src0, Src1, _spill_c3_to_src1, lower, sq,
)
from concourse.dve_uop import DveOpSpec

RC0 = -0.23548383
RC1 = 2.00161239
RC2 = 1.00011986
AB0 = RC0 * float(np.sqrt(RC2))
AB1 = RC1 * float(np.sqrt(RC2))

# deg-5 odd minimax sin on arctan range (max num abs err ~3e-5)
S5C0 = 0.9997329
S5C1 = -0.16575311
S5C2 = 0.00754758


def _notf(a):
    return (~np.asarray(a, np.float32).view(np.int32)).view(np.float32)


def _ref_recipt(in0, in1, c0, c1, c2):
    s = np.asarray(in0, np.float32) + np.asarray(in1, np.float32)
    y0 = _notf(s) * np.float32(c0)
    return (y0 * (np.float32(c1) - s * y0)).astype(np.float32)


def _spec_recipt():
    s = Bin(AluOp.ADD, Src0, Src1)
    n = Bin(AluOp.BITWISE_NOT, s, s)
    y0 = n * C0
    y1 = y0 * (C1 - s * y0)
    return Spec(body=y1, reference=_ref_recipt)


def _ref_sinm(in0, in1, c0, c1, c2):
    # masked sin(theta): (theta*mask) * P(theta^2); mask carries the 0.5
    th = np.asarray(in0, np.float32)
    m = np.asarray(in1, np.float32)
    u = th * th
    p = np.float32(c0) + u * (np.float32(c1) + u * np.float32(c2))
    return ((th * m) * p).astype(np.float32)


def _spec_sinm():
    m0 = Src0 * Src1
    u = sq(Src0)
    p = C0 + u * (C1 + u * C2)
    return Spec(body=m0 * p, reference=_ref_sinm)


def _ref_sina(in0, in1, c0, c1, c2):
    # 0.5 + theta*P(theta^2) with 0.5-scaled coeffs; in1 = [P,1] 0.5 const
    th = np.asarray(in0, np.float32)
    u = th * th
    p = np.float32(c0) + u * (np.float32(c1) + u * np.float32(c2))
    return (np.asarray(in1, np.float32) + th * p).astype(np.float32)


def _spec_sina():
    u = sq(Src0)
    p = C0 + u * (C1 + u * C2)
    return Spec(body=_spill_c3_to_src1(C3 + Src0 * p), reference=_ref_sina)


def _register(name, spec, subdim=False):
    for op in dve_ops.OPS:
        if op.name == name:
            return op
    opcode = dve_ops._CUSTOM_DVE_ROW_BASE + len(dve_ops.OPS)
    assert opcode < 0x20
    rd1_en = dve_ops.has_src1(spec)
    shas = {}
    for ver in ("v3", "v4"):
        try:
            uops = lower(spec, ver=ver)
            shas[ver] = DveOpSpec(name=name, opcode=opcode, uops=uops,
                                  rd1_en=rd1_en).sha(ver)
        except Exception:
            pass
    op = DveOp(name, spec, subdim, uops_sha=shas)
    dve_ops.OPS.append(op)
    dve_ops._SUB_OPCODE_FOR_NAME[name] = opcode
    dve_ops.CUSTOM_DVE_SPECS[name] = spec
    return op


RECIPT_ANT = _register("RECIPT_ANT", _spec_recipt())
SINM_ANT = _register("SINM_ANT", _spec_sinm())
SINA_ANT = _register("SINA_ANT", _spec_sina())

# diag-tile geometry: for dd = jt-4*tau in 0..3 the i-window of the
# [128 j, 512 i] o_ps block is [IL[dd], IL[dd]+WD[dd]); within it
# dist = c - p (window col c, partition p).
IL = [0, 128, 256, 384]
WD = [512, 384, 256, 128]

# --------------------------------------------------------------------------
# Problem constants
# --------------------------------------------------------------------------
B, T, C, H, D = 2, 2048, 1024, 16, 64
NCORES = 8
HPC = 4                 # heads per core
SCALE = 1.0 / 8.0       # 1/sqrt(D)
DEXT = D + 4            # q/k + [islope_hi, islope_lo, 1, 1] / [1, 1, jhi, jlo]
NT = T // 512           # 4 i-chunks of 512
NJT = T // 128          # 16 j-tiles of 128
DFAR = 17.0             # |alibi| beyond which num < ~1e-3 (cut distance)

F32 = mybir.dt.float32
F16 = mybir.dt.float16
AF = mybir.ActivationFunctionType

_PROG = {}


def _ceil64(x):
    return int(np.ceil(x / 64.0)) * 64


def _plan(slopes=None):
    """Head->slot assignment and per-slot computed-width tables."""
    if slopes is None:
        start = 2.0 ** (-8.0 / H)
        slopes = np.asarray([start ** (i + 1) for i in range(H)], np.float32)
    slopes = np.asarray(slopes, np.float32)
    d = DFAR / np.maximum(np.abs(slopes), 1e-12)
    order = np.argsort(-d, kind="stable")
    quartets = [order[4 * s:4 * s + 4].tolist() for s in range(4)]
    Dm = [int(np.ceil(max(float(d[h]) for h in quartets[s])))
          for s in range(4)]
    weoff = {}
    for s in range(4):
        for tau in range(NT):
            for jt in range(4 * tau):
                we = 128 * jt + 128 + Dm[s] - 512 * tau
                weoff[(s, tau, jt)] = min(max(_ceil64(we), 0), 512)
    wed = [[min(WD[dd], _ceil64(128 + Dm[s])) for dd in range(4)]
           for s in range(4)]
    key = tuple(Dm)
    return quartets, Dm, weoff, wed, key


# --------------------------------------------------------------------------
# Device program (identical on all 8 cores)
# --------------------------------------------------------------------------
def _build_program(reps=1, slopes=None):
    import os
    dbg = os.environ.get("BASSDBG", "")
    nopool = os.environ.get("BASSNOPOOL", "1") == "1"
    nofuse = os.environ.get("BASSNOFUSE", "") == "1"
    sccopy = os.environ.get("BASSSCCOPY", "1") == "1"
    defer = os.environ.get("BASSDEFER", "1") == "1"
    _, Dm, weoff, wed, key = _plan(slopes)
    cache_key = (reps, key, dbg, nopool, nofuse, sccopy, defer)
    if cache_key in _PROG:
        return _PROG[cache_key]

    nc = bacc.Bacc("TRN2", target_bir_lowering=False, debug=False,
                   num_devices=NCORES)

    d_xT = nc.dram_tensor("xT", [NT, 2, 128, 4, 512], F16,
                          kind="ExternalInput")
    d_wqk = nc.dram_tensor("wqk", [128, 8, 512], F16, kind="ExternalInput")
    d_wv = nc.dram_tensor("wv", [128, 8, 256], F16, kind="ExternalInput")
    d_wo = nc.dram_tensor("wo", [128, 2, 1024], F16, kind="ExternalInput")
    d_qext = nc.dram_tensor("qext", [4, 4, T], F16, kind="ExternalInput")
    d_kext = nc.dram_tensor("kext", [4, 4, T], F16, kind="ExternalInput")
    d_tri = nc.dram_tensor("trimask", [128, 4, 512], F16,
                           kind="ExternalInput")
    d_tail = nc.dram_tensor("tail", [64, 4, T], F16, kind="ExternalInput")
    d_out = nc.dram_tensor("out_p", [T, C], F16, kind="ExternalOutput")

    with TileContext(nc) as tc:
        with (
            tc.tile_pool(name="const", bufs=1) as cpool,
            tc.tile_pool(name="ew", bufs=6) as ew,
            tc.tile_pool(name="osb", bufs=3) as osb,
            tc.tile_pool(name="acc", bufs=2, space="PSUM") as accp,
            tc.tile_pool(name="ps", bufs=4, space="PSUM") as psp,
            tc.tile_pool(name="pso", bufs=2, space="PSUM") as psop,
        ):
            # ---------------- persistent tensors ----------------
            wqk_sb = cpool.tile([128, 8, 512], F16, tag="wqk")
            wv_sb = cpool.tile([128, 8, 256], F16, tag="wv")
            wo_sb = cpool.tile([128, 2, 1024], F16, tag="wo")
            q_all = cpool.tile([128, HPC, T], F16, tag="q_all")
            k_all = cpool.tile([128, HPC, T], F16, tag="k_all")
            # v double-buffered by iteration parity: lets the next
            # iteration's V projection start while this iteration's last
            # pairs still read the current buffer.
            v_sb = cpool.tile([128, 2, NJT, HPC * 128], F16, tag="v_sb")
            o_all = cpool.tile([128, 2, T], F16, tag="o_all")
            tri = cpool.tile([128, 4, 512], F16, tag="tri")
            tail_sb = cpool.tile([64, 4, T], F16, tag="tail")
            xsb = cpool.tile([128, NT, 2, 4, 512], F16, tag="xsb")

            nc.sync.dma_start(wqk_sb[:], d_wqk[:])
            nc.sync.dma_start(wv_sb[:], d_wv[:])
            nc.sync.dma_start(wo_sb[:], d_wo[:])
            nc.sync.dma_start(tri[:], d_tri[:])
            nc.sync.dma_start(tail_sb[:], d_tail[:])
            for tau in range(NT):
                for half in range(2):
                    nc.sync.dma_start(xsb[:, tau, half], d_xT[tau, half])

            # constants: hoisted out of the timing rep-loop (idempotent).
            # ext rows:   even slots at rows [64:68) (matmul reads [0:68)),
            # odd slots at rows [60:64) with zeros in [0:60) (matmul reads
            # [0:128) — ldweights requires partition base 0 for >32 rows).
            for h in range(HPC):
                if h % 2 == 1:
                    nc.vector.memset(q_all[0:64, h, :], 0.0)
                    nc.vector.memset(k_all[0:64, h, :], 0.0)
                base = 64 if h % 2 == 0 else 60
                nc.sync.dma_start(q_all[base:base + 4, h, :], d_qext[:, h, :])
                nc.sync.dma_start(k_all[base:base + 4, h, :], d_kext[:, h, :])

            half = cpool.tile([128, 1], F32, tag="half")
            nc.vector.memset(half[:], 0.5)
            if dbg == "noew":
                num_const = cpool.tile([128, 512], F16, tag="numc")
                nc.vector.memset(num_const[:], 0.001)
            # o_all is read by the deferred phase3(3) before the first
            # iteration writes it — initialize to keep the garbage finite.
            nc.vector.memset(o_all[:], 0.001)
            # ones columns of V_ext in cols 0:64 of each head's group, so
            # the P@V denominator lands at o_ps partitions [0:64) (custom DVE
            # ops require partition base 0 on their input).
            v4 = v_sb[:].rearrange("p b t (h e) -> p b t h e", e=128)
            nc.gpsimd.memset(v4[:, :, :, :, 0:64], 1.0)

            import contextlib
            # 2 unrolled bodies per hw-loop iteration (v ping-pong):
            # effective iteration count is 2*ceil(reps/2) for reps > 1.
            loop_ctx = (tc.For_i(0, (reps + 1) // 2, 1) if reps > 1
                        else contextlib.nullcontext())
            if True:
              # Issue order per round tau:
              #   pairA(tau) -> phase3(tau-1) -> pairB(tau) -> phase1(tau+1)
              # The PE-only projection segments are sandwiched between
              # attention pairs, so the elementwise engines drain their
              # attention backlog while the PE runs projections, instead of
              # idling per round (phase1(0) is the prologue, phase3(NT-1)
              # the epilogue).
              LA = 5  # QK lookahead within a pair (psp ring bounds it too)

              # build-time per-engine load model (ns) for assigning each
              # tile's sin+finisher to ScalarE/DVE/Pool (greedy min-max).
              LOAD = {"sc": 0.0, "dve": 0.0, "pool": 0.0}

              def p1_v_unit(tau, ttl, vb):
                  xa = xsb[:, tau, 0]
                  xb = xsb[:, tau, 1]
                  tt = 4 * tau + ttl
                  accv = accp.tile([128, 256], F32, tag="acc", name="accv")
                  for kt in range(8):
                      xt = xa if kt < 4 else xb
                      nc.tensor.matmul(
                          accv[:], xt[:, kt % 4, 128 * ttl:128 * ttl + 128],
                          wv_sb[:, kt, :],
                          start=(kt == 0), stop=(kt == 7))
                  nc.vector.tensor_copy(
                      out=v4[:, vb, tt, :, 64:128],
                      in_=accv[:].rearrange("p (h e) -> p h e", e=64))
                  LOAD["dve"] += 327

              def p1_qk_unit(tau, mt):
                  ts = slice(512 * tau, 512 * tau + 512)
                  xa = xsb[:, tau, 0]
                  xb = xsb[:, tau, 1]
                  acc = accp.tile([128, 512], F32, tag="acc", name="acc")
                  for kt in range(8):
                      xt = xa if kt < 4 else xb
                      nc.tensor.matmul(
                          acc[:], wqk_sb[:, kt, 128 * mt:128 * mt + 128],
                          xt[:, kt % 4, :],
                          start=(kt == 0), stop=(kt == 7))
                  dst = q_all if mt < 2 else k_all
                  h0 = 2 * (mt % 2)
                  for (p0, hh) in ((0, h0), (64, h0 + 1)):
                      if sccopy and LOAD["sc"] + 594 <= LOAD["dve"] + 594:
                          nc.scalar.activation(
                              dst[p0:p0 + 64, hh, ts],
                              acc[p0:p0 + 64, :], AF.Copy)
                          LOAD["sc"] += 594
                      else:
                          nc.vector.tensor_copy(
                              out=dst[p0:p0 + 64, hh, ts],
                              in_=acc[p0:p0 + 64, :])
                          LOAD["dve"] += 594

              def phase1_units(tau, vb):
                  # V first: the next round's diag/tri matmuls need it
                  return ([lambda ttl=ttl: p1_v_unit(tau, ttl, vb)
                           for ttl in range(4)]
                          + [lambda mt=mt: p1_qk_unit(tau, mt)
                             for mt in (0, 2, 1, 3)])

              def p3_unit(tau, ttl, oc):
                  tt = 4 * tau + ttl
                  acc = accp.tile([128, 512], F32, tag="acc", name="acc3")
                  for half in range(2):
                      nc.tensor.matmul(
                          acc[:],
                          o_all[:, half, 128 * tt:128 * tt + 128],
                          wo_sb[:, half, 512 * oc:512 * oc + 512],
                          start=(half == 0), stop=(half == 1))
                  ot = osb.tile([128, 512], F16, tag="ot", name="ot")
                  if sccopy and LOAD["sc"] + 594 <= LOAD["dve"] + 594:
                      nc.scalar.activation(ot[:], acc[:], AF.Copy)
                      LOAD["sc"] += 594
                  else:
                      nc.vector.tensor_copy(out=ot[:], in_=acc[:])
                      LOAD["dve"] += 594
                  if dbg != "nodma":
                      nc.sync.dma_start(
                          d_out[128 * tt:128 * tt + 128,
                                512 * oc:512 * oc + 512],
                          ot[:])

              def phase3_units(tau):
                  return [lambda ttl=ttl, oc=oc: p3_unit(tau, ttl, oc)
                          for ttl in range(4) for oc in range(2)]

              def phase1(tau, vb=0):
                  for u in phase1_units(tau, vb):
                      u()

              def phase3(tau):
                  for u in phase3_units(tau):
                      u()

              def attn_pair(tau, hA, hB, inter=(), vb=0):
                  i0 = 512 * tau
                  isl = slice(i0, i0 + 512)
                  njt = 4 * (tau + 1)
                  blocks = []
                  for h in (hA, hB):
                      blocks.append(dict(
                          h=h,
                          hb=0, hk=(DEXT if h % 2 == 0 else 128),
                          o_ps=psop.tile([128, 512], F32, tag="pso",
                                         name="o_ps")))

                  def geom(h, n):
                      dd = n - 4 * tau
                      if dd >= 0:
                          return IL[dd], wed[h][dd]
                      return 0, weoff[(h, tau, n)]

                  def qk(bi, n):
                      blk = blocks[bi]
                      il, wd = geom(blk["h"], n)
                      x_ps = psp.tile([128, 512], F32, tag="ps", name="x_ps")
                      nc.tensor.matmul(
                          x_ps[:, 0:wd],
                          k_all[blk["hb"]:blk["hb"] + blk["hk"],
                                blk["h"], 128 * n:128 * n + 128],
                          q_all[blk["hb"]:blk["hb"] + blk["hk"],
                                blk["h"], i0 + il:i0 + il + wd],
                          start=True, stop=True)
                      return x_ps

                  # wide tiles first, narrow tiles last: the round's tail is
                  # then short elementwise chains, minimizing the in-order PE
                  # bubble at the pair boundary.
                  sched = [(bi, n) for n in range(njt) for bi in (0, 1)
                           if geom(blocks[bi]["h"], n)[1] > 0]
                  sched.sort(key=lambda s: -geom(blocks[s[0]]["h"], s[1])[1])
                  total = {0: 0, 1: 0}
                  for bi, n in sched:
                      total[bi] += 1

                  tiles = {}
                  for idx in range(min(LA, len(sched))):
                      tiles[sched[idx]] = qk(*sched[idx])
                  # group openers: constant +0.5*mask part of the diag tiles
                  # (no elementwise dependency -> PE never waits)
                  for bi in (0, 1):
                      h = blocks[bi]["h"]
                      for dd in range(4):
                          jt = 4 * tau + dd
                          nc.tensor.matmul(
                              blocks[bi]["o_ps"][:, IL[dd]:IL[dd] + WD[dd]],
                              v_sb[:, vb, jt, 128 * h:128 * h + 128],
                              tri[:, h, 0:WD[dd]],
                              start=(dd == 0), stop=False,
                              skip_group_check=True)
                  def denom(bi):
                      # emitted as soon as the block's last PV lands: frees
                      # the psop bank early for the next pair.
                      h = blocks[bi]["h"]
                      o_ps = blocks[bi]["o_ps"]
                      rsb = ew.tile([64, 512], F32, tag="rsb", name="rsb")
                      nc.vector._custom_dve(
                          RECIPT_ANT, out=rsb[:], in0=o_ps[0:64, :],
                          in1=tail_sb[:, h, isl], s0=AB0, s1=AB1)
                      nc.vector.tensor_mul(
                          out=o_all[64 * (h % 2):64 * (h % 2) + 64,
                                    h // 2, isl],
                          in0=o_ps[64:128, :], in1=rsb[:])
                      LOAD["dve"] += 2 * 593

                  done = {0: 0, 1: 0}
                  k_inter = 0
                  for idx, (bi, jt) in enumerate(sched):
                      # pace the interleaved PE-only projection units so the
                      # elementwise engines keep receiving fresh scores
                      # instead of starving during contiguous projection
                      # bursts.
                      want = (idx * len(inter)) // max(len(sched) - 1, 1)
                      while k_inter < want:
                          inter[k_inter]()
                          k_inter += 1
                      if idx + LA < len(sched):
                          tiles[sched[idx + LA]] = qk(*sched[idx + LA])
                      x_ps = tiles.pop((bi, jt))
                      blk = blocks[bi]
                      h = blk["h"]
                      dd = jt - 4 * tau  # >= 0 on diagonal block
                      il, wd = geom(h, jt)
                      if dbg == "noew":
                          num = num_const
                      else:
                          num = ew.tile([128, 512], F16, tag="num",
                                        name="num")
                          # x/sqrt(1+x^2) = sin(arctan(x)): ScalarE arctan,
                          # then either a ScalarE Sin pass + DVE/Pool f16
                          # finisher (affine / tri-mask mul), or a single
                          # fused deg-5 sin-poly DVE op with the finisher
                          # folded in.  Greedy min-max over the modeled
                          # engine loads picks per tile.  x_ps is freed
                          # right after the arctan pass.
                          at = ew.tile([128, 512], F32, tag="at", name="at")
                          nc.scalar.activation(at[:, 0:wd], x_ps[:, 0:wd],
                                               AF.Arctan)
                          LOAD["sc"] += wd * 1.043 + 60
                          c_sin = wd * 1.043 + 60
                          c_fin_dve = (wd * 0.52 + 60) if dd >= 0 else (
                              wd * 0.30 + 60)
                          c_fin_pool = wd * 2.48 + 95
                          c_fused = wd * 1.043 + 125
                          mA = max(LOAD["sc"] + c_sin,
                                   LOAD["dve"] + c_fin_dve, LOAD["pool"])
                          mB = max(LOAD["sc"] + c_sin, LOAD["dve"],
                                   LOAD["pool"] + c_fin_pool)
                          mC = max(LOAD["sc"], LOAD["dve"] + c_fused,
                                   LOAD["pool"])
                          if nopool:
                              mB = float("inf")
                          if nofuse:
                              mC = float("inf")
                          if mC <= mA and mC <= mB:
                              # fused DVE sin (+mask / +0.5 affine)
                              LOAD["dve"] += c_fused
                              if dd >= 0:
                                  nc.vector._custom_dve(
                                      SINM_ANT, out=num[:, 0:wd],
                                      in0=at[:, 0:wd], in1=tri[:, h, 0:wd],
                                      s0=S5C0, s1=S5C1, imm2=S5C2)
                              else:
                                  nc.vector._custom_dve(
                                      SINA_ANT, out=num[:, 0:wd],
                                      in0=at[:, 0:wd], in1=half[:],
                                      s0=0.5 * S5C0, s1=0.5 * S5C1,
                                      imm2=0.5 * S5C2)
                          else:
                              un = ew.tile([128, 512], F16, tag="un",
                                           name="un")
                              nc.scalar.activation(un[:, 0:wd], at[:, 0:wd],
                                                   AF.Sin)
                              LOAD["sc"] += c_sin
                              eng = nc.vector if mA <= mB else nc.gpsimd
                              LOAD["dve" if mA <= mB else "pool"] += (
                                  c_fin_dve if mA <= mB else c_fin_pool)
                              if dd < 0:
                                  eng.tensor_scalar(
                                      out=num[:, 0:wd], in0=un[:, 0:wd],
                                      scalar1=0.5, scalar2=0.5,
                                      op0=mybir.AluOpType.mult,
                                      op1=mybir.AluOpType.add)
                              else:
                                  eng.tensor_mul(
                                      out=num[:, 0:wd], in0=un[:, 0:wd],
                                      in1=tri[:, h, 0:wd])
                      done[bi] += 1
                      if dbg != "nopv":
                          nc.tensor.matmul(
                              blk["o_ps"][:, il:il + wd],
                              v_sb[:, vb, jt, 128 * h:128 * h + 128],
                              num[:, 0:wd],
                              start=False, stop=(done[bi] == total[bi]),
                              skip_group_check=True)
                          # denominators (rows 0:64, replicated by the ones
                          # columns; custom-DVE requires partition base 0).
                          # RECIPT adds the host-precomputed dropped-tail
                          # mass and inverts in one op.
                          if done[bi] == total[bi]:
                              denom(bi)
                  while k_inter < len(inter):
                      inter[k_inter]()
                      k_inter += 1

              if dbg in ("proj_only", "nodma", "p1only"):
                  with loop_ctx:
                      phase1(0)
                      for tau in range(NT):
                          if tau >= 1 and dbg != "p1only":
                              phase3(tau - 1)
                          if tau + 1 < NT:
                              phase1(tau + 1)
                      if dbg != "p1only":
                          phase3(NT - 1)
              elif defer:
                  # Uniform rounds: round tau interleaves phase3 of the
                  # PREVIOUS round ((tau-1) mod 4: round 0 drains the prior
                  # iteration's round 3) and phase1 of the NEXT round
                  # ((tau+1) mod 4: round 3 prefetches the next iteration's
                  # round 0 into the other v buffer).  The first iteration's
                  # deferred phase3(3) runs on initialized garbage and is
                  # overwritten; the final phase3(3) runs after the loop.
                  phase1(0, 0)  # prologue, outside the hw loop
                  nbody = 2 if reps > 1 else 1
                  with loop_ctx:
                      for vb in range(nbody):
                          for tau in range(NT):
                              units = phase3_units((tau - 1) % NT)
                              nvb = ((1 - vb) % nbody
                                     if tau + 1 == NT else vb)
                              units += phase1_units((tau + 1) % NT, nvb)
                              nh = (len(units) + 1) // 2
                              attn_pair(tau, 0, 3, units[:nh], vb)
                              attn_pair(tau, 1, 2, units[nh:], vb)
                  phase3(NT - 1)  # epilogue: the last iteration's round 3
              else:
                  with loop_ctx:
                      phase1(0, 0)
                      for tau in range(NT):
                          units = []
                          if tau >= 1:
                              units += phase3_units(tau - 1)
                          if tau + 1 < NT:
                              units += phase1_units(tau + 1, 0)
                          nh = (len(units) + 1) // 2
                          attn_pair(tau, 0, 3, units[:nh], 0)
                          attn_pair(tau, 1, 2, units[nh:], 0)
                      phase3(NT - 1)

    nc.compile()
    _PROG[cache_key] = nc
    return nc


# --------------------------------------------------------------------------
# Host-side input preparation
# --------------------------------------------------------------------------
def _split2(v):
    v = v.astype(np.float32)
    p1 = v.astype(np.float16).astype(np.float32)
    p2 = (v - p1).astype(np.float16)
    return p1.astype(np.float16), p2


def _computed_mask_for_slot(Dm, weoff_s, wed_s):
    """[T, T] bool over (i, j): True where the pair is computed on-device."""
    keep = np.zeros((T, T), dtype=bool)
    for tau in range(NT):
        i0 = 512 * tau
        for jt in range(4 * tau):
            we = weoff_s[(tau, jt)]
            if we > 0:
                keep[i0:i0 + we, 128 * jt:128 * jt + 128] = True
        for dd in range(4):
            we_d = wed_s[dd]
            j0 = i0 + 128 * dd
            c = np.arange(we_d)[:, None]
            p = np.arange(128)[None, :]
            m = (c - p >= 0) & (c - p <= Dm)
            keep[i0 + IL[dd]:i0 + IL[dd] + we_d, j0:j0 + 128] = m
    return keep


def _host_prep(x, w_qkv, w_out, alibi_slopes):
    x = np.asarray(x, np.float32)
    w_qkv = np.asarray(w_qkv, np.float32)
    w_out = np.asarray(w_out, np.float32)
    slopes = np.asarray(alibi_slopes, np.float32)
    quartets, Dm, weoff, wed, _ = _plan(slopes)

    iarr = np.arange(T, dtype=np.float32)
    # per-slot 0.5-valued tri masks: 0.5 iff 0 <= c - p <= Dm[s]
    p = np.arange(128)[:, None]
    c = np.arange(512)[None, :]
    trimask = np.zeros((128, 4, 512), np.float16)
    for s in range(4):
        trimask[:, s, :] = (((c - p) >= 0) & ((c - p) <= Dm[s])).astype(
            np.float16) * np.float16(0.5)

    # dropped-pair masks + per-(slot-geometry) distance weights are shared
    # across cores; the tail itself depends on the head's slope.
    rel = np.arange(T)[:, None] - np.arange(T)[None, :]
    causal_valid = rel >= 0
    dropped_s = []
    for s in range(4):
        weoff_s = {(tau, jt): weoff[(s, tau, jt)]
                   for tau in range(NT) for jt in range(4 * tau)}
        keep = _computed_mask_for_slot(Dm[s], weoff_s, wed[s])
        dropped_s.append((~keep) & causal_valid)

    def tail_for(s, slope):
        d = np.abs(rel).astype(np.float32) * np.float32(slope)
        num_a = 0.5 * (1.0 - d / np.sqrt(1.0 + d * d))
        return (num_a * dropped_s[s]).sum(axis=1).astype(np.float32)  # [T]

    in_maps = []
    for cc in range(NCORES):
        b = cc // 4
        g = cc % 4
        heads = [quartets[s][g] for s in range(HPC)]

        # pre-swizzled to the SBUF tile layout: [tau, half, p, k, t]
        xTf = np.ascontiguousarray(x[b].T).astype(np.float16)
        xT = np.ascontiguousarray(
            xTf.reshape(2, 4, 128, 4, 512).transpose(3, 0, 2, 1, 4))

        q_rows = np.concatenate(
            [w_qkv[64 * h:64 * h + 64] for h in heads], axis=0) * SCALE
        k_rows = np.concatenate(
            [w_qkv[C + 64 * h:C + 64 * h + 64] for h in heads], axis=0)
        qk_rows = np.concatenate([q_rows, k_rows], axis=0)  # [512, 1024]
        wqk = np.ascontiguousarray(
            qk_rows.T.reshape(8, 128, 512).transpose(1, 0, 2)).astype(np.float16)

        v_rows = np.concatenate(
            [w_qkv[2 * C + 64 * h:2 * C + 64 * h + 64] for h in heads], axis=0)
        wv = np.ascontiguousarray(
            v_rows.T.reshape(8, 128, 256).transpose(1, 0, 2)).astype(np.float16)

        Wg = np.concatenate(
            [w_out[:, 64 * h:64 * h + 64] for h in heads], axis=1)  # [1024,256]
        wo = np.ascontiguousarray(
            Wg.T.reshape(2, 128, 1024).transpose(1, 0, 2)).astype(np.float16)

        qext = np.zeros((4, HPC, T), np.float16)
        kext = np.zeros((4, HPC, T), np.float16)
        tail = np.zeros((64, HPC, T), np.float16)
        for j, h in enumerate(heads):
            sl = float(slopes[h])
            ihi, ilo = _split2(-iarr * sl)
            jhi, jlo = _split2(iarr * sl)
            qext[0, j] = ihi
            qext[1, j] = ilo
            qext[2, j] = 1.0
            qext[3, j] = 1.0
            kext[0, j] = 1.0
            kext[1, j] = 1.0
            kext[2, j] = jhi
            kext[3, j] = jlo
            tail[:, j, :] = tail_for(j, sl)[None, :].astype(np.float16)

        in_maps.append({
            "xT": xT, "wqk": wqk, "wv": wv, "wo": wo,
            "qext": qext, "kext": kext, "trimask": trimask, "tail": tail,
        })
    return in_maps


def _assemble(partials):
    out = np.zeros((B, T, C), np.float32)
    for c in range(NCORES):
        out[c // 4] += partials[c]
    return out.astype(np.float32)


def kernel(x, w_qkv, w_out, alibi_slopes):
    nc = _build_program(slopes=alibi_slopes)
    in_maps = _host_prep(x, w_qkv, w_out, alibi_slopes)
    res = run_bass_kernel_spmd(nc, in_maps, core_ids=list(range(NCORES)))
    return _assemble([r["out_p"] for r in res.results])


# revision 49
# speedup vs baseline: 1.9660x; 1.5767x over previous
"""Trainium2 Bass kernel for nn_AlgebraicAttention (8-core SPMD).

Sharding: core c -> batch b = c//4, head quartet column g = c%4.  Heads are
sorted by ALiBi reach d_h = 17/slope_h (descending) and grouped into four
rank-quartets; program head-slot s on core g runs head quartets[s][g].  Every
core executes the identical program with identical tile geometry (required:
one SPMD program for all 8 cores); per-core data (weights, tail constants)
carries the head differences.  Each core computes its 4 heads' attention and
a partial out-projection; the host sums the 8 partials.

Math notes:
  - scores^T layout [j (keys, partitions), i (queries, free)].
  - ALiBi (j-i)*slope folded into the QK^T contraction via 4 extra f16 rows
    (hi/lo splits of -i*slope and j*slope).
  - geometric cut: per slot, only (i,j) pairs with dist = i-j <= Dm[slot]
    (Dm = ceil(17/min_slope_of_quartet)) are computed.  Off-diag tiles
    narrow to we columns (64-rounded); fully-far tiles drop entirely; diag
    windows narrow to wed and the per-slot 0.5-valued tri mask also zeroes
    pairs with dist > Dm.  The dropped pairs' contribution to the softmax
    DENOMINATOR is systematic (all-positive); it is precomputed on the host
    as tail(h, i) = sum_dropped num(alibi) and added back inside the
    reciprocal DVE op (RECIPT).  The dropped numerator contribution is a
    random-sign sum of O(1e-3) weights -> statistically negligible.
  - rational softmax numerator num = 0.5*(1 + x/sqrt(1+x^2)) computed as
    sin(arctan(x)) in 2 table-based ScalarE passes + a cheap DVE f16 affine
    (off-diag) or tri-mask multiply (diag).
  - the +0.5*mask constant part of diag num comes from triangular matmuls
    in P@V (also supplying the denominator's diag mass); the denominator
    itself is a ones-column in the P@V matmul, inverted with the 1-Newton
    RECIPT op (max rel err ~0.17%) that also adds the tail constant.
"""

import numpy as np

import concourse.bass as bass
import concourse.mybir as mybir
from concourse import bacc
from concourse.tile import TileContext
from concourse.bass_utils import run_bass_kernel_spmd

# --------------------------------------------------------------------------
# Custom DVE op: out = approx 1/(Src0 + Src1) (1-Newton, ~0.17% max rel err)
# --------------------------------------------------------------------------
import concourse.dve_ops as dve_ops
from concourse.dve_ops import DveOp
from concourse.dve_spec import (
    AluOp, Bin, C0, C1, C2, C3, Spec, Src0, Src1, _spill_c3_to_src1, lower, sq,
)
from concourse.dve_uop import DveOpSpec

RC0 = -0.23548383
RC1 = 2.00161239
RC2 = 1.00011986
AB0 = RC0 * float(np.sqrt(RC2))
AB1 = RC1 * float(np.sqrt(RC2))

# deg-5 odd minimax sin on arctan range (max num abs err ~3e-5)
S5C0 = 0.9997329
S5C1 = -0.16575311
S5C2 = 0.00754758


def _notf(a):
    return (~np.asarray(a, np.float32).view(np.int32)).view(np.float32)


def _ref_recipt(in0, in1, c0, c1, c2):
    s = np.asarray(in0, np.float32) + np.asarray(in1, np.float32)
    y0 = _notf(s) * np.float32(c0)
    return (y0 * (np.float32(c1) - s * y0)).astype(np.float32)


def _spec_recipt():
    s = Bin(AluOp.ADD, Src0, Src1)
    n = Bin(AluOp.BITWISE_NOT, s, s)
    y0 = n * C0
    y1 = y0 * (C1 - s * y0)
    return Spec(body=y1, reference=_ref_recipt)


def _ref_sinf(in0, in1, c0, c1, c2):
    # full masked num: tri * (1 + theta*P(theta^2)); tri carries the 0.5
    th = np.asarray(in0, np.float32)
    m = np.asarray(in1, np.float32)
    u = th * th
    p = np.float32(c0) + u * (np.float32(c1) + u * np.float32(c2))
    return (m * (1.0 + th * p)).astype(np.float32)


def _spec_sinf():
    from concourse.dve_spec import One
    u = sq(Src0)
    p = C0 + u * (C1 + u * C2)
    s = Src0 * p
    return Spec(body=Src1 * Bin(AluOp.ADD, One, s), reference=_ref_sinf)


def _ref_trim(in0, in1, c0, c1, c2):
    # diag finisher for the ScalarE-sin path: tri * (1 + sin)
    return (np.asarray(in1, np.float32)
            * (1.0 + np.asarray(in0, np.float32))).astype(np.float32)


def _spec_trim():
    from concourse.dve_spec import One
    return Spec(body=Src1 * Bin(AluOp.ADD, One, Src0), reference=_ref_trim)


def _ref_sina(in0, in1, c0, c1, c2):
    # 0.5 + theta*P(theta^2) with 0.5-scaled coeffs; in1 = [P,1] 0.5 const
    th = np.asarray(in0, np.float32)
    u = th * th
    p = np.float32(c0) + u * (np.float32(c1) + u * np.float32(c2))
    return (np.asarray(in1, np.float32) + th * p).astype(np.float32)


def _spec_sina():
    u = sq(Src0)
    p = C0 + u * (C1 + u * C2)
    return Spec(body=_spill_c3_to_src1(C3 + Src0 * p), reference=_ref_sina)


def _register(name, spec, subdim=False):
    for op in dve_ops.OPS:
        if op.name == name:
            return op
    opcode = dve_ops._CUSTOM_DVE_ROW_BASE + len(dve_ops.OPS)
    assert opcode < 0x20
    rd1_en = dve_ops.has_src1(spec)
    shas = {}
    for ver in ("v3", "v4"):
        try:
            uops = lower(spec, ver=ver)
            shas[ver] = DveOpSpec(name=name, opcode=opcode, uops=uops,
                                  rd1_en=rd1_en).sha(ver)
        except Exception:
            pass
    op = DveOp(name, spec, subdim, uops_sha=shas)
    dve_ops.OPS.append(op)
    dve_ops._SUB_OPCODE_FOR_NAME[name] = opcode
    dve_ops.CUSTOM_DVE_SPECS[name] = spec
    return op


RECIPT_ANT = _register("RECIPT_ANT", _spec_recipt())
SINF_ANT = _register("SINF_ANT", _spec_sinf())
SINA_ANT = _register("SINA_ANT", _spec_sina())
TRIM_ANT = _register("TRIM_ANT", _spec_trim())

# diag-tile geometry: for dd = jt-4*tau in 0..3 the i-window of the
# [128 j, 512 i] o_ps block is [IL[dd], IL[dd]+WD[dd]); within it
# dist = c - p (window col c, partition p).
IL = [0, 128, 256, 384]
WD = [512, 384, 256, 128]

# --------------------------------------------------------------------------
# Problem constants
# --------------------------------------------------------------------------
B, T, C, H, D = 2, 2048, 1024, 16, 64
NCORES = 8
HPC = 4                 # heads per core
SCALE = 1.0 / 8.0       # 1/sqrt(D)
DEXT = D + 4            # q/k + [islope_hi, islope_lo, 1, 1] / [1, 1, jhi, jlo]
NT = T // 512           # 4 i-chunks of 512
NJT = T // 128          # 16 j-tiles of 128
DFAR = 17.0             # |alibi| beyond which num < ~1e-3 (cut distance)

F32 = mybir.dt.float32
F16 = mybir.dt.float16
AF = mybir.ActivationFunctionType

_PROG = {}


def _ceil64(x):
    return int(np.ceil(x / 64.0)) * 64


def _plan(slopes=None):
    """Head->slot assignment and per-slot computed-width tables."""
    if slopes is None:
        start = 2.0 ** (-8.0 / H)
        slopes = np.asarray([start ** (i + 1) for i in range(H)], np.float32)
    slopes = np.asarray(slopes, np.float32)
    d = DFAR / np.maximum(np.abs(slopes), 1e-12)
    order = np.argsort(-d, kind="stable")
    quartets = [order[4 * s:4 * s + 4].tolist() for s in range(4)]
    Dm = [int(np.ceil(max(float(d[h]) for h in quartets[s])))
          for s in range(4)]
    weoff = {}
    for s in range(4):
        for tau in range(NT):
            for jt in range(4 * tau):
                we = 128 * jt + 128 + Dm[s] - 512 * tau
                weoff[(s, tau, jt)] = min(max(_ceil64(we), 0), 512)
    wed = [[min(WD[dd], _ceil64(128 + Dm[s])) for dd in range(4)]
           for s in range(4)]
    key = tuple(Dm)
    return quartets, Dm, weoff, wed, key


# --------------------------------------------------------------------------
# Device program (identical on all 8 cores)
# --------------------------------------------------------------------------
def _build_program(reps=1, slopes=None):
    import os
    dbg = os.environ.get("BASSDBG", "")
    nopool = os.environ.get("BASSNOPOOL", "1") == "1"
    nofuse = os.environ.get("BASSNOFUSE", "") == "1"
    sccopy = os.environ.get("BASSSCCOPY", "1") == "1"
    defer = os.environ.get("BASSDEFER", "1") == "1"
    unroll = int(os.environ.get("BASSUNROLL", "4"))
    _, Dm, weoff, wed, key = _plan(slopes)
    cache_key = (reps, key, dbg, nopool, nofuse, sccopy, defer, unroll)
    if cache_key in _PROG:
        return _PROG[cache_key]

    nc = bacc.Bacc("TRN2", target_bir_lowering=False, debug=False,
                   num_devices=NCORES)

    d_xT = nc.dram_tensor("xT", [NT, 2, 128, 4, 512], F16,
                          kind="ExternalInput")
    d_wqk = nc.dram_tensor("wqk", [128, 8, 512], F16, kind="ExternalInput")
    d_wv = nc.dram_tensor("wv", [128, 8, 256], F16, kind="ExternalInput")
    d_wo = nc.dram_tensor("wo", [128, 2, 1024], F16, kind="ExternalInput")
    d_qext = nc.dram_tensor("qext", [4, 4, T], F16, kind="ExternalInput")
    d_kext = nc.dram_tensor("kext", [4, 4, T], F16, kind="ExternalInput")
    d_tri = nc.dram_tensor("trimask", [128, 4, 512], F16,
                           kind="ExternalInput")
    d_tail = nc.dram_tensor("tail", [1, 4, T], F16, kind="ExternalInput")
    d_out = nc.dram_tensor("out_p", [T, C], F16, kind="ExternalOutput")

    with TileContext(nc) as tc:
        with (
            tc.tile_pool(name="const", bufs=1) as cpool,
            tc.tile_pool(name="ew", bufs=6) as ew,
            tc.tile_pool(name="osb", bufs=3) as osb,
            tc.tile_pool(name="acc", bufs=2, space="PSUM") as accp,
            tc.tile_pool(name="ps", bufs=4, space="PSUM") as psp,
            tc.tile_pool(name="pso", bufs=2, space="PSUM") as psop,
        ):
            # ---------------- persistent tensors ----------------
            wqk_sb = cpool.tile([128, 8, 512], F16, tag="wqk")
            wv_sb = cpool.tile([128, 8, 256], F16, tag="wv")
            wo_sb = cpool.tile([128, 2, 1024], F16, tag="wo")
            q_all = cpool.tile([128, HPC, T], F16, tag="q_all")
            k_all = cpool.tile([128, HPC, T], F16, tag="k_all")
            # v double-buffered by iteration parity: lets the next
            # iteration's V projection start while this iteration's last
            # pairs still read the current buffer.
            v_sb = cpool.tile([128, 2, NJT, HPC * 128], F16, tag="v_sb")
            o_all = cpool.tile([128, 2, T], F16, tag="o_all")
            tri = cpool.tile([128, 4, 512], F16, tag="tri")
            tail_sb = cpool.tile([1, 4, T], F16, tag="tail")
            xsb = cpool.tile([128, NT, 2, 4, 512], F16, tag="xsb")

            nc.sync.dma_start(wqk_sb[:], d_wqk[:])
            nc.sync.dma_start(wv_sb[:], d_wv[:])
            nc.sync.dma_start(wo_sb[:], d_wo[:])
            nc.sync.dma_start(tri[:], d_tri[:])
            nc.sync.dma_start(tail_sb[:], d_tail[:])
            for tau in range(NT):
                for half in range(2):
                    nc.sync.dma_start(xsb[:, tau, half], d_xT[tau, half])

            # constants: hoisted out of the timing rep-loop (idempotent).
            # ext rows:   even slots at rows [64:68) (matmul reads [0:68)),
            # odd slots at rows [60:64) with zeros in [0:60) (matmul reads
            # [0:128) — ldweights requires partition base 0 for >32 rows).
            for h in range(HPC):
                if h % 2 == 1:
                    nc.vector.memset(q_all[0:64, h, :], 0.0)
                    nc.vector.memset(k_all[0:64, h, :], 0.0)
                base = 64 if h % 2 == 0 else 60
                nc.sync.dma_start(q_all[base:base + 4, h, :], d_qext[:, h, :])
                nc.sync.dma_start(k_all[base:base + 4, h, :], d_kext[:, h, :])

            half = cpool.tile([128, 1], F32, tag="half")
            nc.vector.memset(half[:], 0.5)
            # [1,128] selector for the o_ps opener: ones over the
            # denominator partitions, zeros over the value partitions.
            dcol = cpool.tile([1, 128], F16, tag="dcol")
            nc.vector.memset(dcol[:, 0:64], 1.0)
            nc.vector.memset(dcol[:, 64:128], 0.0)
            if dbg == "noew":
                num_const = cpool.tile([128, 512], F16, tag="numc")
                nc.vector.memset(num_const[:], 0.001)
            # o_all is read by the deferred phase3(3) before the first
            # iteration writes it — initialize to keep the garbage finite.
            nc.vector.memset(o_all[:], 0.001)
            # ones columns of V_ext in cols 0:64 of each head's group, so
            # the P@V denominator lands at o_ps partitions [0:64) (custom DVE
            # ops require partition base 0 on their input).
            v4 = v_sb[:].rearrange("p b t (h e) -> p b t h e", e=128)
            nc.gpsimd.memset(v4[:, :, :, :, 0:64], 1.0)

            import contextlib
            # `unroll` bodies per hw-loop iteration (v ping-pong):
            # amortizes the For_i all-engine barrier; effective iteration
            # count is unroll*ceil(reps/unroll) for reps > 1.
            loop_ctx = (tc.For_i(0, (reps + unroll - 1) // unroll, 1)
                        if reps > 1 else contextlib.nullcontext())
            if True:
              # Issue order per round tau:
              #   pairA(tau) -> phase3(tau-1) -> pairB(tau) -> phase1(tau+1)
              # The PE-only projection segments are sandwiched between
              # attention pairs, so the elementwise engines drain their
              # attention backlog while the PE runs projections, instead of
              # idling per round (phase1(0) is the prologue, phase3(NT-1)
              # the epilogue).
              LA = 5  # QK lookahead within a pair (psp ring bounds it too)

              # build-time per-engine load model (ns) for assigning each
              # tile's sin+finisher to ScalarE/DVE/Pool (greedy min-max).
              LOAD = {"sc": 0.0, "dve": 0.0, "pool": 0.0}

              def p1_v_unit(tau, ttl, vb):
                  xa = xsb[:, tau, 0]
                  xb = xsb[:, tau, 1]
                  tt = 4 * tau + ttl
                  accv = accp.tile([128, 256], F32, tag="acc", name="accv")
                  for kt in range(8):
                      xt = xa if kt < 4 else xb
                      nc.tensor.matmul(
                          accv[:], xt[:, kt % 4, 128 * ttl:128 * ttl + 128],
                          wv_sb[:, kt, :],
                          start=(kt == 0), stop=(kt == 7))
                  nc.vector.tensor_copy(
                      out=v4[:, vb, tt, :, 64:128],
                      in_=accv[:].rearrange("p (h e) -> p h e", e=64))
                  LOAD["dve"] += 327

              def p1_qk_unit(tau, mt):
                  ts = slice(512 * tau, 512 * tau + 512)
                  xa = xsb[:, tau, 0]
                  xb = xsb[:, tau, 1]
                  acc = accp.tile([128, 512], F32, tag="acc", name="acc")
                  for kt in range(8):
                      xt = xa if kt < 4 else xb
                      nc.tensor.matmul(
                          acc[:], wqk_sb[:, kt, 128 * mt:128 * mt + 128],
                          xt[:, kt % 4, :],
                          start=(kt == 0), stop=(kt == 7))
                  dst = q_all if mt < 2 else k_all
                  h0 = 2 * (mt % 2)
                  for (p0, hh) in ((0, h0), (64, h0 + 1)):
                      if sccopy and LOAD["sc"] + 594 <= LOAD["dve"] + 594:
                          nc.scalar.activation(
                              dst[p0:p0 + 64, hh, ts],
                              acc[p0:p0 + 64, :], AF.Copy)
                          LOAD["sc"] += 594
                      else:
                          nc.vector.tensor_copy(
                              out=dst[p0:p0 + 64, hh, ts],
                              in_=acc[p0:p0 + 64, :])
                          LOAD["dve"] += 594

              def phase1_units(tau, vb):
                  # V first: the next round's diag/tri matmuls need it
                  return ([lambda ttl=ttl: p1_v_unit(tau, ttl, vb)
                           for ttl in range(4)]
                          + [lambda mt=mt: p1_qk_unit(tau, mt)
                             for mt in (0, 2, 1, 3)])

              def p3_unit(tau, ttl, oc):
                  tt = 4 * tau + ttl
                  acc = accp.tile([128, 512], F32, tag="acc", name="acc3")
                  for half in range(2):
                      nc.tensor.matmul(
                          acc[:],
                          o_all[:, half, 128 * tt:128 * tt + 128],
                          wo_sb[:, half, 512 * oc:512 * oc + 512],
                          start=(half == 0), stop=(half == 1))
                  ot = osb.tile([128, 512], F16, tag="ot", name="ot")
                  if sccopy and LOAD["sc"] + 594 <= LOAD["dve"] + 594:
                      nc.scalar.activation(ot[:], acc[:], AF.Copy)
                      LOAD["sc"] += 594
                  else:
                      nc.vector.tensor_copy(out=ot[:], in_=acc[:])
                      LOAD["dve"] += 594
                  if dbg != "nodma":
                      nc.sync.dma_start(
                          d_out[128 * tt:128 * tt + 128,
                                512 * oc:512 * oc + 512],
                          ot[:])

              def phase3_units(tau):
                  return [lambda ttl=ttl, oc=oc: p3_unit(tau, ttl, oc)
                          for ttl in range(4) for oc in range(2)]

              def phase1(tau, vb=0):
                  for u in phase1_units(tau, vb):
                      u()

              def phase3(tau):
                  for u in phase3_units(tau):
                      u()

              def attn_pair(tau, hA, hB, inter=(), vb=0):
                  i0 = 512 * tau
                  isl = slice(i0, i0 + 512)
                  njt = 4 * (tau + 1)
                  blocks = []
                  for h in (hA, hB):
                      blocks.append(dict(
                          h=h,
                          hb=0, hk=(DEXT if h % 2 == 0 else 128),
                          o_ps=psop.tile([128, 512], F32, tag="pso",
                                         name="o_ps")))

                  def geom(h, n):
                      dd = n - 4 * tau
                      if dd >= 0:
                          return IL[dd], wed[h][dd]
                      return 0, weoff[(h, tau, n)]

                  def qk(bi, n):
                      blk = blocks[bi]
                      il, wd = geom(blk["h"], n)
                      x_ps = psp.tile([128, 512], F32, tag="ps", name="x_ps")
                      nc.tensor.matmul(
                          x_ps[:, 0:wd],
                          k_all[blk["hb"]:blk["hb"] + blk["hk"],
                                blk["h"], 128 * n:128 * n + 128],
                          q_all[blk["hb"]:blk["hb"] + blk["hk"],
                                blk["h"], i0 + il:i0 + il + wd],
                          start=True, stop=True)
                      return x_ps

                  # wide tiles first, narrow tiles last: the round's tail is
                  # then short elementwise chains, minimizing the in-order PE
                  # bubble at the pair boundary.
                  sched = [(bi, n) for n in range(njt) for bi in (0, 1)
                           if geom(blocks[bi]["h"], n)[1] > 0]
                  sched.sort(key=lambda s: -geom(blocks[s[0]]["h"], s[1])[1])
                  total = {0: 0, 1: 0}
                  for bi, n in sched:
                      total[bi] += 1

                  tiles = {}
                  for idx in range(min(LA, len(sched))):
                      tiles[sched[idx]] = qk(*sched[idx])
                  # group opener: one 1-row matmul zero-fills each o_ps and
                  # injects the host-precomputed dropped-tail mass into the
                  # denominator rows (dcol = [1s x64 | 0s x64]).
                  for bi in (0, 1):
                      h = blocks[bi]["h"]
                      nc.tensor.matmul(
                          blocks[bi]["o_ps"][:],
                          dcol[:], tail_sb[0:1, h, isl],
                          start=True, stop=False,
                          skip_group_check=True)
                  def denom(bi):
                      # emitted as soon as the block's last PV lands: frees
                      # the psop bank early for the next pair.  The tail
                      # mass is already in the denominator via the opener.
                      h = blocks[bi]["h"]
                      o_ps = blocks[bi]["o_ps"]
                      rsb = ew.tile([64, 512], F32, tag="rsb", name="rsb")
                      nc.vector.reciprocal_approx_fast(out=rsb[:],
                                                       in_=o_ps[0:64, :])
                      nc.vector.tensor_mul(
                          out=o_all[64 * (h % 2):64 * (h % 2) + 64,
                                    h // 2, isl],
                          in0=o_ps[64:128, :], in1=rsb[:])
                      LOAD["dve"] += 2 * 593

                  done = {0: 0, 1: 0}
                  k_inter = 0
                  for idx, (bi, jt) in enumerate(sched):
                      # pace the interleaved PE-only projection units so the
                      # elementwise engines keep receiving fresh scores
                      # instead of starving during contiguous projection
                      # bursts.
                      want = (idx * len(inter)) // max(len(sched) - 1, 1)
                      while k_inter < want:
                          inter[k_inter]()
                          k_inter += 1
                      if idx + LA < len(sched):
                          tiles[sched[idx + LA]] = qk(*sched[idx + LA])
                      x_ps = tiles.pop((bi, jt))
                      blk = blocks[bi]
                      h = blk["h"]
                      dd = jt - 4 * tau  # >= 0 on diagonal block
                      il, wd = geom(h, jt)
                      if dbg == "noew":
                          num = num_const
                      else:
                          num = ew.tile([128, 512], F16, tag="num",
                                        name="num")
                          # x/sqrt(1+x^2) = sin(arctan(x)): ScalarE arctan,
                          # then either a ScalarE Sin pass + DVE/Pool f16
                          # finisher (affine / tri-mask mul), or a single
                          # fused deg-5 sin-poly DVE op with the finisher
                          # folded in.  Greedy min-max over the modeled
                          # engine loads picks per tile.  x_ps is freed
                          # right after the arctan pass.
                          at = ew.tile([128, 512], F32, tag="at", name="at")
                          nc.scalar.activation(at[:, 0:wd], x_ps[:, 0:wd],
                                               AF.Arctan)
                          LOAD["sc"] += wd * 1.043 + 60
                          c_sin = wd * 1.043 + 60
                          c_fin_dve = (wd * 0.52 + 60) if dd >= 0 else (
                              wd * 0.30 + 60)
                          c_fin_pool = wd * 2.48 + 95
                          c_fused = wd * 1.043 + 125
                          mA = max(LOAD["sc"] + c_sin,
                                   LOAD["dve"] + c_fin_dve, LOAD["pool"])
                          mB = max(LOAD["sc"] + c_sin, LOAD["dve"],
                                   LOAD["pool"] + c_fin_pool)
                          mC = max(LOAD["sc"], LOAD["dve"] + c_fused,
                                   LOAD["pool"])
                          if nopool:
                              mB = float("inf")
                          if nofuse:
                              mC = float("inf")
                          if mC <= mA and mC <= mB:
                              # fused DVE sin (full masked num / +0.5 affine)
                              LOAD["dve"] += c_fused
                              if dd >= 0:
                                  nc.vector._custom_dve(
                                      SINF_ANT, out=num[:, 0:wd],
                                      in0=at[:, 0:wd], in1=tri[:, h, 0:wd],
                                      s0=S5C0, s1=S5C1, imm2=S5C2)
                              else:
                                  nc.vector._custom_dve(
                                      SINA_ANT, out=num[:, 0:wd],
                                      in0=at[:, 0:wd], in1=half[:],
                                      s0=0.5 * S5C0, s1=0.5 * S5C1,
                                      imm2=0.5 * S5C2)
                          else:
                              un = ew.tile([128, 512], F16, tag="un",
                                           name="un")
                              nc.scalar.activation(un[:, 0:wd], at[:, 0:wd],
                                                   AF.Sin)
                              LOAD["sc"] += c_sin
                              LOAD["dve"] += c_fin_dve
                              if dd < 0:
                                  nc.vector.tensor_scalar(
                                      out=num[:, 0:wd], in0=un[:, 0:wd],
                                      scalar1=0.5, scalar2=0.5,
                                      op0=mybir.AluOpType.mult,
                                      op1=mybir.AluOpType.add)
                              else:
                                  nc.vector._custom_dve(
                                      TRIM_ANT, out=num[:, 0:wd],
                                      in0=un[:, 0:wd],
                                      in1=tri[:, h, 0:wd])
                      done[bi] += 1
                      if dbg != "nopv":
                          nc.tensor.matmul(
                              blk["o_ps"][:, il:il + wd],
                              v_sb[:, vb, jt, 128 * h:128 * h + 128],
                              num[:, 0:wd],
                              start=False, stop=(done[bi] == total[bi]),
                              skip_group_check=True)
                          # denominators (rows 0:64, replicated by the ones
                          # columns; custom-DVE requires partition base 0).
                          # RECIPT adds the host-precomputed dropped-tail
                          # mass and inverts in one op.
                          if done[bi] == total[bi]:
                              denom(bi)
                  while k_inter < len(inter):
                      inter[k_inter]()
                      k_inter += 1

              if dbg in ("proj_only", "nodma", "p1only"):
                  with loop_ctx:
                      phase1(0)
                      for tau in range(NT):
                          if tau >= 1 and dbg != "p1only":
                              phase3(tau - 1)
                          if tau + 1 < NT:
                              phase1(tau + 1)
                      if dbg != "p1only":
                          phase3(NT - 1)
              elif defer:
                  # Uniform rounds: round tau interleaves phase3 of the
                  # PREVIOUS round ((tau-1) mod 4: round 0 drains the prior
                  # iteration's round 3) and phase1 of the NEXT round
                  # ((tau+1) mod 4: round 3 prefetches the next iteration's
                  # round 0 into the other v buffer).  The first iteration's
                  # deferred phase3(3) runs on initialized garbage and is
                  # overwritten; the final phase3(3) runs after the loop.
                  phase1(0, 0)  # prologue, outside the hw loop
                  nbody = (unroll if reps > 1
                           else int(os.environ.get("BASSBODIES", "1")))
                  with loop_ctx:
                      for body in range(nbody):
                          vb = body % 2
                          for tau in range(NT):
                              units = phase3_units((tau - 1) % NT)
                              nvb = ((vb + 1) % 2
                                     if tau + 1 == NT else vb)
                              units += phase1_units((tau + 1) % NT, nvb)
                              nh = (len(units) + 1) // 2
                              attn_pair(tau, 0, 3, units[:nh], vb)
                              attn_pair(tau, 1, 2, units[nh:], vb)
                  phase3(NT - 1)  # epilogue: the last iteration's round 3
              else:
                  with loop_ctx:
                      phase1(0, 0)
                      for tau in range(NT):
                          units = []
                          if tau >= 1:
                              units += phase3_units(tau - 1)
                          if tau + 1 < NT:
                              units += phase1_units(tau + 1, 0)
                          nh = (len(units) + 1) // 2
                          attn_pair(tau, 0, 3, units[:nh], 0)
                          attn_pair(tau, 1, 2, units[nh:], 0)
                      phase3(NT - 1)

    nc.compile()
    _PROG[cache_key] = nc
    return nc


# --------------------------------------------------------------------------
# Host-side input preparation
# --------------------------------------------------------------------------
def _split2(v):
    v = v.astype(np.float32)
    p1 = v.astype(np.float16).astype(np.float32)
    p2 = (v - p1).astype(np.float16)
    return p1.astype(np.float16), p2


def _computed_mask_for_slot(Dm, weoff_s, wed_s):
    """[T, T] bool over (i, j): True where the pair is computed on-device."""
    keep = np.zeros((T, T), dtype=bool)
    for tau in range(NT):
        i0 = 512 * tau
        for jt in range(4 * tau):
            we = weoff_s[(tau, jt)]
            if we > 0:
                keep[i0:i0 + we, 128 * jt:128 * jt + 128] = True
        for dd in range(4):
            we_d = wed_s[dd]
            j0 = i0 + 128 * dd
            c = np.arange(we_d)[:, None]
            p = np.arange(128)[None, :]
            m = (c - p >= 0) & (c - p <= Dm)
            keep[i0 + IL[dd]:i0 + IL[dd] + we_d, j0:j0 + 128] = m
    return keep


def _host_prep(x, w_qkv, w_out, alibi_slopes):
    x = np.asarray(x, np.float32)
    w_qkv = np.asarray(w_qkv, np.float32)
    w_out = np.asarray(w_out, np.float32)
    slopes = np.asarray(alibi_slopes, np.float32)
    quartets, Dm, weoff, wed, _ = _plan(slopes)

    iarr = np.arange(T, dtype=np.float32)
    # per-slot 0.5-valued tri masks: 0.5 iff 0 <= c - p <= Dm[s]
    p = np.arange(128)[:, None]
    c = np.arange(512)[None, :]
    trimask = np.zeros((128, 4, 512), np.float16)
    for s in range(4):
        trimask[:, s, :] = (((c - p) >= 0) & ((c - p) <= Dm[s])).astype(
            np.float16) * np.float16(0.5)

    # dropped-pair masks + per-(slot-geometry) distance weights are shared
    # across cores; the tail itself depends on the head's slope.
    rel = np.arange(T)[:, None] - np.arange(T)[None, :]
    causal_valid = rel >= 0
    dropped_s = []
    for s in range(4):
        weoff_s = {(tau, jt): weoff[(s, tau, jt)]
                   for tau in range(NT) for jt in range(4 * tau)}
        keep = _computed_mask_for_slot(Dm[s], weoff_s, wed[s])
        dropped_s.append((~keep) & causal_valid)

    def tail_for(s, slope):
        d = np.abs(rel).astype(np.float32) * np.float32(slope)
        num_a = 0.5 * (1.0 - d / np.sqrt(1.0 + d * d))
        return (num_a * dropped_s[s]).sum(axis=1).astype(np.float32)  # [T]

    in_maps = []
    for cc in range(NCORES):
        b = cc // 4
        g = cc % 4
        heads = [quartets[s][g] for s in range(HPC)]

        # pre-swizzled to the SBUF tile layout: [tau, half, p, k, t]
        xTf = np.ascontiguousarray(x[b].T).astype(np.float16)
        xT = np.ascontiguousarray(
            xTf.reshape(2, 4, 128, 4, 512).transpose(3, 0, 2, 1, 4))

        q_rows = np.concatenate(
            [w_qkv[64 * h:64 * h + 64] for h in heads], axis=0) * SCALE
        k_rows = np.concatenate(
            [w_qkv[C + 64 * h:C + 64 * h + 64] for h in heads], axis=0)
        qk_rows = np.concatenate([q_rows, k_rows], axis=0)  # [512, 1024]
        wqk = np.ascontiguousarray(
            qk_rows.T.reshape(8, 128, 512).transpose(1, 0, 2)).astype(np.float16)

        v_rows = np.concatenate(
            [w_qkv[2 * C + 64 * h:2 * C + 64 * h + 64] for h in heads], axis=0)
        wv = np.ascontiguousarray(
            v_rows.T.reshape(8, 128, 256).transpose(1, 0, 2)).astype(np.float16)

        Wg = np.concatenate(
            [w_out[:, 64 * h:64 * h + 64] for h in heads], axis=1)  # [1024,256]
        wo = np.ascontiguousarray(
            Wg.T.reshape(2, 128, 1024).transpose(1, 0, 2)).astype(np.float16)

        qext = np.zeros((4, HPC, T), np.float16)
        kext = np.zeros((4, HPC, T), np.float16)
        tail = np.zeros((1, HPC, T), np.float16)
        for j, h in enumerate(heads):
            sl = float(slopes[h])
            ihi, ilo = _split2(-iarr * sl)
            jhi, jlo = _split2(iarr * sl)
            qext[0, j] = ihi
            qext[1, j] = ilo
            qext[2, j] = 1.0
            qext[3, j] = 1.0
            kext[0, j] = 1.0
            kext[1, j] = 1.0
            kext[2, j] = jhi
            kext[3, j] = jlo
            tail[0, j, :] = tail_for(j, sl).astype(np.float16)

        in_maps.append({
            "xT": xT, "wqk": wqk, "wv": wv, "wo": wo,
            "qext": qext, "kext": kext, "trimask": trimask, "tail": tail,
        })
    return in_maps


def _assemble(partials):
    out = np.zeros((B, T, C), np.float32)
    for c in range(NCORES):
        out[c // 4] += partials[c]
    return out.astype(np.float32)


def kernel(x, w_qkv, w_out, alibi_slopes):
    nc = _build_program(slopes=alibi_slopes)
    in_maps = _host_prep(x, w_qkv, w_out, alibi_slopes)
    res = run_bass_kernel_spmd(nc, in_maps, core_ids=list(range(NCORES)))
    return _assemble([r["out_p"] for r in res.results])


# revision 56
# speedup vs baseline: 2.0440x; 1.0397x over previous
"""Trainium2 Bass kernel for nn_AlgebraicAttention (8-core SPMD).

Sharding: core c -> batch b = c//4, head quartet column g = c%4.  Heads are
sorted by ALiBi reach d_h = 17/slope_h (descending) and grouped into four
rank-quartets; program head-slot s on core g runs head quartets[s][g].  Every
core executes the identical program with identical tile geometry (required:
one SPMD program for all 8 cores); per-core data (weights, tail constants)
carries the head differences.  Each core computes its 4 heads' attention and
a partial out-projection; the host sums the 8 partials.

Math notes:
  - scores^T layout [j (keys, partitions), i (queries, free)].
  - ALiBi (j-i)*slope folded into the QK^T contraction via 4 extra f16 rows
    (hi/lo splits of -i*slope and j*slope).
  - geometric cut: per slot, only (i,j) pairs with dist = i-j <= Dm[slot]
    (Dm = ceil(17/min_slope_of_quartet)) are computed.  Off-diag tiles
    narrow to we columns (64-rounded); fully-far tiles drop entirely; diag
    windows narrow to wed and the per-slot 0.5-valued tri mask also zeroes
    pairs with dist > Dm.  The dropped pairs' contribution to the softmax
    DENOMINATOR is systematic (all-positive); it is precomputed on the host
    as tail(h, i) = sum_dropped num(alibi) and injected by the per-block
    o_ps opener matmul (1-row: [1s|0s] x tail row).  The dropped numerator
    contribution is a random-sign sum of O(1e-3) weights -> negligible.
  - rational softmax numerator num = 0.5*(1 + x/sqrt(1+x^2)) computed as
    sin(arctan(x)): ScalarE arctan, then per-tile either a fused deg-5
    sin-poly DVE op (SINF with tri mask folded / SINA with +0.5 affine) or
    a ScalarE Sin pass + cheap DVE finisher, chosen by a build-time greedy
    min-max over modeled engine loads.
  - the denominator is a ones-column in the P@V matmul, inverted with
    reciprocal_approx_fast; phase1/phase3 projection matmul groups are
    interleaved between attention tiles (deferred across round and
    iteration boundaries, v double-buffered) so no engine starves; the
    For_i all-engine barrier is amortized by a 4-body unroll.
"""

import numpy as np

import concourse.bass as bass
import concourse.mybir as mybir
from concourse import bacc
from concourse.tile import TileContext
from concourse.bass_utils import run_bass_kernel_spmd

# --------------------------------------------------------------------------
# Custom DVE op: out = approx 1/(Src0 + Src1) (1-Newton, ~0.17% max rel err)
# --------------------------------------------------------------------------
import concourse.dve_ops as dve_ops
from concourse.dve_ops import DveOp
from concourse.dve_spec import (
    AluOp, Bin, C0, C1, C2, C3, Spec, Src0, Src1, _spill_c3_to_src1, lower, sq,
)
from concourse.dve_uop import DveOpSpec

RC0 = -0.23548383
RC1 = 2.00161239
RC2 = 1.00011986
AB0 = RC0 * float(np.sqrt(RC2))
AB1 = RC1 * float(np.sqrt(RC2))

# deg-5 odd minimax sin on arctan range (max num abs err ~3e-5)
S5C0 = 0.9997329
S5C1 = -0.16575311
S5C2 = 0.00754758


def _notf(a):
    return (~np.asarray(a, np.float32).view(np.int32)).view(np.float32)


def _ref_recipt(in0, in1, c0, c1, c2):
    s = np.asarray(in0, np.float32) + np.asarray(in1, np.float32)
    y0 = _notf(s) * np.float32(c0)
    return (y0 * (np.float32(c1) - s * y0)).astype(np.float32)


def _spec_recipt():
    s = Bin(AluOp.ADD, Src0, Src1)
    n = Bin(AluOp.BITWISE_NOT, s, s)
    y0 = n * C0
    y1 = y0 * (C1 - s * y0)
    return Spec(body=y1, reference=_ref_recipt)


def _ref_sinf(in0, in1, c0, c1, c2):
    # full masked num: tri * (1 + theta*P(theta^2)); tri carries the 0.5
    th = np.asarray(in0, np.float32)
    m = np.asarray(in1, np.float32)
    u = th * th
    p = np.float32(c0) + u * (np.float32(c1) + u * np.float32(c2))
    return (m * (1.0 + th * p)).astype(np.float32)


def _spec_sinf():
    from concourse.dve_spec import One
    u = sq(Src0)
    p = C0 + u * (C1 + u * C2)
    s = Src0 * p
    return Spec(body=Src1 * Bin(AluOp.ADD, One, s), reference=_ref_sinf)


def _ref_trim(in0, in1, c0, c1, c2):
    # diag finisher for the ScalarE-sin path: tri * (1 + sin)
    return (np.asarray(in1, np.float32)
            * (1.0 + np.asarray(in0, np.float32))).astype(np.float32)


def _spec_trim():
    from concourse.dve_spec import One
    return Spec(body=Src1 * Bin(AluOp.ADD, One, Src0), reference=_ref_trim)


def _ref_sina(in0, in1, c0, c1, c2):
    # 0.5 + theta*P(theta^2) with 0.5-scaled coeffs; in1 = [P,1] 0.5 const
    th = np.asarray(in0, np.float32)
    u = th * th
    p = np.float32(c0) + u * (np.float32(c1) + u * np.float32(c2))
    return (np.asarray(in1, np.float32) + th * p).astype(np.float32)


def _spec_sina():
    u = sq(Src0)
    p = C0 + u * (C1 + u * C2)
    return Spec(body=_spill_c3_to_src1(C3 + Src0 * p), reference=_ref_sina)


def _register(name, spec, subdim=False):
    for op in dve_ops.OPS:
        if op.name == name:
            return op
    opcode = dve_ops._CUSTOM_DVE_ROW_BASE + len(dve_ops.OPS)
    assert opcode < 0x20
    rd1_en = dve_ops.has_src1(spec)
    shas = {}
    for ver in ("v3", "v4"):
        try:
            uops = lower(spec, ver=ver)
            shas[ver] = DveOpSpec(name=name, opcode=opcode, uops=uops,
                                  rd1_en=rd1_en).sha(ver)
        except Exception:
            pass
    op = DveOp(name, spec, subdim, uops_sha=shas)
    dve_ops.OPS.append(op)
    dve_ops._SUB_OPCODE_FOR_NAME[name] = opcode
    dve_ops.CUSTOM_DVE_SPECS[name] = spec
    return op


RECIPT_ANT = _register("RECIPT_ANT", _spec_recipt())
SINF_ANT = _register("SINF_ANT", _spec_sinf())
SINA_ANT = _register("SINA_ANT", _spec_sina())
TRIM_ANT = _register("TRIM_ANT", _spec_trim())

# diag-tile geometry: for dd = jt-4*tau in 0..3 the i-window of the
# [128 j, 512 i] o_ps block is [IL[dd], IL[dd]+WD[dd]); within it
# dist = c - p (window col c, partition p).
IL = [0, 128, 256, 384]
WD = [512, 384, 256, 128]

# --------------------------------------------------------------------------
# Problem constants
# --------------------------------------------------------------------------
B, T, C, H, D = 2, 2048, 1024, 16, 64
NCORES = 8
HPC = 4                 # heads per core
SCALE = 1.0 / 8.0       # 1/sqrt(D)
DEXT = D + 4            # q/k + [islope_hi, islope_lo, 1, 1] / [1, 1, jhi, jlo]
NT = T // 512           # 4 i-chunks of 512
NJT = T // 128          # 16 j-tiles of 128
DFAR = 11.0             # |alibi| cut distance: beyond it num < ~2e-3 and
                        # the host tail constant covers the dropped mass

F32 = mybir.dt.float32
F16 = mybir.dt.float16
AF = mybir.ActivationFunctionType

_PROG = {}


def _ceil64(x):
    return int(np.ceil(x / 64.0)) * 64


def _plan(slopes=None):
    """Head->slot assignment and per-slot computed-width tables."""
    import os
    dfar = float(os.environ.get("BASSDFAR", str(DFAR)))
    if slopes is None:
        start = 2.0 ** (-8.0 / H)
        slopes = np.asarray([start ** (i + 1) for i in range(H)], np.float32)
    slopes = np.asarray(slopes, np.float32)
    d = dfar / np.maximum(np.abs(slopes), 1e-12)
    order = np.argsort(-d, kind="stable")
    quartets = [order[4 * s:4 * s + 4].tolist() for s in range(4)]
    Dm = [int(np.ceil(max(float(d[h]) for h in quartets[s])))
          for s in range(4)]
    weoff = {}
    for s in range(4):
        for tau in range(NT):
            for jt in range(4 * tau):
                we = 128 * jt + 128 + Dm[s] - 512 * tau
                weoff[(s, tau, jt)] = min(max(_ceil64(we), 0), 512)
    wed = [[min(WD[dd], _ceil64(128 + Dm[s])) for dd in range(4)]
           for s in range(4)]
    key = tuple(Dm)
    return quartets, Dm, weoff, wed, key


# --------------------------------------------------------------------------
# Device program (identical on all 8 cores)
# --------------------------------------------------------------------------
def _build_program(reps=1, slopes=None):
    import os
    dbg = os.environ.get("BASSDBG", "")
    nopool = os.environ.get("BASSNOPOOL", "1") == "1"
    nofuse = os.environ.get("BASSNOFUSE", "") == "1"
    sccopy = os.environ.get("BASSSCCOPY", "1") == "1"
    defer = os.environ.get("BASSDEFER", "1") == "1"
    unroll = int(os.environ.get("BASSUNROLL", "4"))
    psum_cfg = (os.environ.get("BASSACC", "2"), os.environ.get("BASSPS", "4"))
    fbias = float(os.environ.get("BASSFB", "1.0"))
    _, Dm, weoff, wed, key = _plan(slopes)
    cache_key = (reps, key, dbg, nopool, nofuse, sccopy, defer, unroll,
                 psum_cfg, fbias)
    if cache_key in _PROG:
        return _PROG[cache_key]

    nc = bacc.Bacc("TRN2", target_bir_lowering=False, debug=False,
                   num_devices=NCORES)

    d_xT = nc.dram_tensor("xT", [NT, 2, 128, 4, 512], F16,
                          kind="ExternalInput")
    d_wqk = nc.dram_tensor("wqk", [128, 8, 512], F16, kind="ExternalInput")
    d_wv = nc.dram_tensor("wv", [128, 8, 256], F16, kind="ExternalInput")
    d_wo = nc.dram_tensor("wo", [128, 2, 1024], F16, kind="ExternalInput")
    d_qext = nc.dram_tensor("qext", [4, 4, T], F16, kind="ExternalInput")
    d_kext = nc.dram_tensor("kext", [4, 4, T], F16, kind="ExternalInput")
    d_tri = nc.dram_tensor("trimask", [128, 4, 512], F16,
                           kind="ExternalInput")
    d_tail = nc.dram_tensor("tail", [1, 4, T], F16, kind="ExternalInput")
    d_out = nc.dram_tensor("out_p", [T, C], F16, kind="ExternalOutput")

    with TileContext(nc) as tc:
        with (
            tc.tile_pool(name="const", bufs=1) as cpool,
            tc.tile_pool(name="ew", bufs=6) as ew,
            tc.tile_pool(name="osb", bufs=3) as osb,
            tc.tile_pool(name="acc", bufs=int(os.environ.get("BASSACC", "2")),
                         space="PSUM") as accp,
            tc.tile_pool(name="ps", bufs=int(os.environ.get("BASSPS", "4")),
                         space="PSUM") as psp,
            tc.tile_pool(name="pso", bufs=2, space="PSUM") as psop,
        ):
            # ---------------- persistent tensors ----------------
            wqk_sb = cpool.tile([128, 8, 512], F16, tag="wqk")
            wv_sb = cpool.tile([128, 8, 256], F16, tag="wv")
            wo_sb = cpool.tile([128, 2, 1024], F16, tag="wo")
            q_all = cpool.tile([128, HPC, T], F16, tag="q_all")
            k_all = cpool.tile([128, HPC, T], F16, tag="k_all")
            # v double-buffered by iteration parity: lets the next
            # iteration's V projection start while this iteration's last
            # pairs still read the current buffer.
            v_sb = cpool.tile([128, 2, NJT, HPC * 128], F16, tag="v_sb")
            o_all = cpool.tile([128, 2, T], F16, tag="o_all")
            tri = cpool.tile([128, 4, 512], F16, tag="tri")
            tail_sb = cpool.tile([1, 4, T], F16, tag="tail")
            xsb = cpool.tile([128, NT, 2, 4, 512], F16, tag="xsb")

            nc.sync.dma_start(wqk_sb[:], d_wqk[:])
            nc.sync.dma_start(wv_sb[:], d_wv[:])
            nc.sync.dma_start(wo_sb[:], d_wo[:])
            nc.sync.dma_start(tri[:], d_tri[:])
            nc.sync.dma_start(tail_sb[:], d_tail[:])
            for tau in range(NT):
                for half in range(2):
                    nc.sync.dma_start(xsb[:, tau, half], d_xT[tau, half])

            # constants: hoisted out of the timing rep-loop (idempotent).
            # ext rows:   even slots at rows [64:68) (matmul reads [0:68)),
            # odd slots at rows [60:64) with zeros in [0:60) (matmul reads
            # [0:128) — ldweights requires partition base 0 for >32 rows).
            for h in range(HPC):
                if h % 2 == 1:
                    nc.vector.memset(q_all[0:64, h, :], 0.0)
                    nc.vector.memset(k_all[0:64, h, :], 0.0)
                base = 64 if h % 2 == 0 else 60
                nc.sync.dma_start(q_all[base:base + 4, h, :], d_qext[:, h, :])
                nc.sync.dma_start(k_all[base:base + 4, h, :], d_kext[:, h, :])

            half = cpool.tile([128, 1], F32, tag="half")
            nc.vector.memset(half[:], 0.5)
            # [1,128] selector for the o_ps opener: ones over the
            # denominator partitions, zeros over the value partitions.
            dcol = cpool.tile([1, 128], F16, tag="dcol")
            nc.vector.memset(dcol[:, 0:64], 1.0)
            nc.vector.memset(dcol[:, 64:128], 0.0)
            if dbg == "noew":
                num_const = cpool.tile([128, 512], F16, tag="numc")
                nc.vector.memset(num_const[:], 0.001)
            # o_all is read by the deferred phase3(3) before the first
            # iteration writes it — initialize to keep the garbage finite.
            nc.vector.memset(o_all[:], 0.001)
            # ones columns of V_ext in cols 0:64 of each head's group, so
            # the P@V denominator lands at o_ps partitions [0:64) (custom DVE
            # ops require partition base 0 on their input).
            v4 = v_sb[:].rearrange("p b t (h e) -> p b t h e", e=128)
            nc.gpsimd.memset(v4[:, :, :, :, 0:64], 1.0)

            import contextlib
            # `unroll` bodies per hw-loop iteration (v ping-pong):
            # amortizes the For_i all-engine barrier; effective iteration
            # count is unroll*ceil(reps/unroll) for reps > 1.
            loop_ctx = (tc.For_i(0, (reps + unroll - 1) // unroll, 1)
                        if reps > 1 else contextlib.nullcontext())
            if True:
              # Issue order per round tau:
              #   pairA(tau) -> phase3(tau-1) -> pairB(tau) -> phase1(tau+1)
              # The PE-only projection segments are sandwiched between
              # attention pairs, so the elementwise engines drain their
              # attention backlog while the PE runs projections, instead of
              # idling per round (phase1(0) is the prologue, phase3(NT-1)
              # the epilogue).
              LA = 5  # QK lookahead within a pair (psp ring bounds it too)

              # build-time per-engine load model (ns) for assigning each
              # tile's sin+finisher to ScalarE/DVE/Pool (greedy min-max).
              LOAD = {"sc": 0.0, "dve": 0.0, "pool": 0.0}

              def p1_v_unit(tau, ttl, vb):
                  xa = xsb[:, tau, 0]
                  xb = xsb[:, tau, 1]
                  tt = 4 * tau + ttl
                  accv = accp.tile([128, 256], F32, tag="acc", name="accv")
                  for kt in range(8):
                      xt = xa if kt < 4 else xb
                      nc.tensor.matmul(
                          accv[:], xt[:, kt % 4, 128 * ttl:128 * ttl + 128],
                          wv_sb[:, kt, :],
                          start=(kt == 0), stop=(kt == 7))
                  nc.vector.tensor_copy(
                      out=v4[:, vb, tt, :, 64:128],
                      in_=accv[:].rearrange("p (h e) -> p h e", e=64))
                  LOAD["dve"] += 327

              def p1_qk_unit(tau, mt):
                  ts = slice(512 * tau, 512 * tau + 512)
                  xa = xsb[:, tau, 0]
                  xb = xsb[:, tau, 1]
                  acc = accp.tile([128, 512], F32, tag="acc", name="acc")
                  for kt in range(8):
                      xt = xa if kt < 4 else xb
                      nc.tensor.matmul(
                          acc[:], wqk_sb[:, kt, 128 * mt:128 * mt + 128],
                          xt[:, kt % 4, :],
                          start=(kt == 0), stop=(kt == 7))
                  dst = q_all if mt < 2 else k_all
                  h0 = 2 * (mt % 2)
                  for (p0, hh) in ((0, h0), (64, h0 + 1)):
                      if sccopy and LOAD["sc"] + 594 <= LOAD["dve"] + 594:
                          nc.scalar.activation(
                              dst[p0:p0 + 64, hh, ts],
                              acc[p0:p0 + 64, :], AF.Copy)
                          LOAD["sc"] += 594
                      else:
                          nc.vector.tensor_copy(
                              out=dst[p0:p0 + 64, hh, ts],
                              in_=acc[p0:p0 + 64, :])
                          LOAD["dve"] += 594

              def phase1_units(tau, vb):
                  # V first: the next round's diag/tri matmuls need it
                  return ([lambda ttl=ttl: p1_v_unit(tau, ttl, vb)
                           for ttl in range(4)]
                          + [lambda mt=mt: p1_qk_unit(tau, mt)
                             for mt in (0, 2, 1, 3)])

              def p3_unit(tau, ttl, oc):
                  tt = 4 * tau + ttl
                  acc = accp.tile([128, 512], F32, tag="acc", name="acc3")
                  for half in range(2):
                      nc.tensor.matmul(
                          acc[:],
                          o_all[:, half, 128 * tt:128 * tt + 128],
                          wo_sb[:, half, 512 * oc:512 * oc + 512],
                          start=(half == 0), stop=(half == 1))
                  ot = osb.tile([128, 512], F16, tag="ot", name="ot")
                  if sccopy and LOAD["sc"] + 594 <= LOAD["dve"] + 594:
                      nc.scalar.activation(ot[:], acc[:], AF.Copy)
                      LOAD["sc"] += 594
                  else:
                      nc.vector.tensor_copy(out=ot[:], in_=acc[:])
                      LOAD["dve"] += 594
                  if dbg != "nodma":
                      nc.sync.dma_start(
                          d_out[128 * tt:128 * tt + 128,
                                512 * oc:512 * oc + 512],
                          ot[:])

              def phase3_units(tau):
                  return [lambda ttl=ttl, oc=oc: p3_unit(tau, ttl, oc)
                          for ttl in range(4) for oc in range(2)]

              def phase1(tau, vb=0):
                  for u in phase1_units(tau, vb):
                      u()

              def phase3(tau):
                  for u in phase3_units(tau):
                      u()

              def attn_pair(tau, hA, hB, inter=(), vb=0):
                  i0 = 512 * tau
                  isl = slice(i0, i0 + 512)
                  njt = 4 * (tau + 1)
                  blocks = []
                  for h in (hA, hB):
                      blocks.append(dict(
                          h=h,
                          hb=0, hk=(DEXT if h % 2 == 0 else 128),
                          o_ps=psop.tile([128, 512], F32, tag="pso",
                                         name="o_ps")))

                  def geom(h, n):
                      dd = n - 4 * tau
                      if dd >= 0:
                          return IL[dd], wed[h][dd]
                      return 0, weoff[(h, tau, n)]

                  def qk(bi, n):
                      blk = blocks[bi]
                      il, wd = geom(blk["h"], n)
                      x_ps = psp.tile([128, 512], F32, tag="ps", name="x_ps")
                      nc.tensor.matmul(
                          x_ps[:, 0:wd],
                          k_all[blk["hb"]:blk["hb"] + blk["hk"],
                                blk["h"], 128 * n:128 * n + 128],
                          q_all[blk["hb"]:blk["hb"] + blk["hk"],
                                blk["h"], i0 + il:i0 + il + wd],
                          start=True, stop=True)
                      return x_ps

                  # wide tiles first, narrow tiles last: the round's tail is
                  # then short elementwise chains, minimizing the in-order PE
                  # bubble at the pair boundary.
                  sched = [(bi, n) for n in range(njt) for bi in (0, 1)
                           if geom(blocks[bi]["h"], n)[1] > 0]
                  sched.sort(key=lambda s: -geom(blocks[s[0]]["h"], s[1])[1])
                  total = {0: 0, 1: 0}
                  for bi, n in sched:
                      total[bi] += 1

                  tiles = {}
                  for idx in range(min(LA, len(sched))):
                      tiles[sched[idx]] = qk(*sched[idx])
                  # group opener: one 1-row matmul zero-fills each o_ps and
                  # injects the host-precomputed dropped-tail mass into the
                  # denominator rows (dcol = [1s x64 | 0s x64]).
                  for bi in (0, 1):
                      h = blocks[bi]["h"]
                      nc.tensor.matmul(
                          blocks[bi]["o_ps"][:],
                          dcol[:], tail_sb[0:1, h, isl],
                          start=True, stop=False,
                          skip_group_check=True)
                  def denom(bi):
                      # emitted as soon as the block's last PV lands: frees
                      # the psop bank early for the next pair.  The tail
                      # mass is already in the denominator via the opener.
                      h = blocks[bi]["h"]
                      o_ps = blocks[bi]["o_ps"]
                      rsb = ew.tile([64, 512], F32, tag="rsb", name="rsb")
                      nc.vector.reciprocal_approx_fast(out=rsb[:],
                                                       in_=o_ps[0:64, :])
                      nc.vector.tensor_mul(
                          out=o_all[64 * (h % 2):64 * (h % 2) + 64,
                                    h // 2, isl],
                          in0=o_ps[64:128, :], in1=rsb[:])
                      LOAD["dve"] += 2 * 593

                  done = {0: 0, 1: 0}
                  k_inter = 0
                  for idx, (bi, jt) in enumerate(sched):
                      # pace the interleaved PE-only projection units so the
                      # elementwise engines keep receiving fresh scores
                      # instead of starving during contiguous projection
                      # bursts.
                      want = (idx * len(inter)) // max(len(sched) - 1, 1)
                      while k_inter < want:
                          inter[k_inter]()
                          k_inter += 1
                      if idx + LA < len(sched):
                          tiles[sched[idx + LA]] = qk(*sched[idx + LA])
                      x_ps = tiles.pop((bi, jt))
                      blk = blocks[bi]
                      h = blk["h"]
                      dd = jt - 4 * tau  # >= 0 on diagonal block
                      il, wd = geom(h, jt)
                      if dbg == "noew":
                          num = num_const
                      else:
                          num = ew.tile([128, 512], F16, tag="num",
                                        name="num")
                          # x/sqrt(1+x^2) = sin(arctan(x)): ScalarE arctan,
                          # then either a ScalarE Sin pass + DVE/Pool f16
                          # finisher (affine / tri-mask mul), or a single
                          # fused deg-5 sin-poly DVE op with the finisher
                          # folded in.  Greedy min-max over the modeled
                          # engine loads picks per tile.  x_ps is freed
                          # right after the arctan pass.
                          at = ew.tile([128, 512], F32, tag="at", name="at")
                          nc.scalar.activation(at[:, 0:wd], x_ps[:, 0:wd],
                                               AF.Arctan)
                          LOAD["sc"] += wd * 1.043 + 60
                          c_sin = wd * 1.043 + 60
                          c_fin_dve = (wd * 0.52 + 60) if dd >= 0 else (
                              wd * 0.30 + 60)
                          c_fin_pool = wd * 2.48 + 95
                          c_fused = (wd * 1.043 * fbias + 125)
                          mA = max(LOAD["sc"] + c_sin,
                                   LOAD["dve"] + c_fin_dve, LOAD["pool"])
                          mB = max(LOAD["sc"] + c_sin, LOAD["dve"],
                                   LOAD["pool"] + c_fin_pool)
                          mC = max(LOAD["sc"], LOAD["dve"] + c_fused,
                                   LOAD["pool"])
                          if nopool:
                              mB = float("inf")
                          if nofuse:
                              mC = float("inf")
                          if mC <= mA and mC <= mB:
                              # fused DVE sin (full masked num / +0.5 affine)
                              LOAD["dve"] += c_fused
                              if dd >= 0:
                                  nc.vector._custom_dve(
                                      SINF_ANT, out=num[:, 0:wd],
                                      in0=at[:, 0:wd], in1=tri[:, h, 0:wd],
                                      s0=S5C0, s1=S5C1, imm2=S5C2)
                              else:
                                  nc.vector._custom_dve(
                                      SINA_ANT, out=num[:, 0:wd],
                                      in0=at[:, 0:wd], in1=half[:],
                                      s0=0.5 * S5C0, s1=0.5 * S5C1,
                                      imm2=0.5 * S5C2)
                          else:
                              un = ew.tile([128, 512], F16, tag="un",
                                           name="un")
                              nc.scalar.activation(un[:, 0:wd], at[:, 0:wd],
                                                   AF.Sin)
                              LOAD["sc"] += c_sin
                              LOAD["dve"] += c_fin_dve
                              if dd < 0:
                                  nc.vector.tensor_scalar(
                                      out=num[:, 0:wd], in0=un[:, 0:wd],
                                      scalar1=0.5, scalar2=0.5,
                                      op0=mybir.AluOpType.mult,
                                      op1=mybir.AluOpType.add)
                              else:
                                  nc.vector._custom_dve(
                                      TRIM_ANT, out=num[:, 0:wd],
                                      in0=un[:, 0:wd],
                                      in1=tri[:, h, 0:wd])
                      done[bi] += 1
                      if dbg != "nopv":
                          nc.tensor.matmul(
                              blk["o_ps"][:, il:il + wd],
                              v_sb[:, vb, jt, 128 * h:128 * h + 128],
                              num[:, 0:wd],
                              start=False, stop=(done[bi] == total[bi]),
                              skip_group_check=True)
                          # denominators (rows 0:64, replicated by the ones
                          # columns; custom-DVE requires partition base 0).
                          # RECIPT adds the host-precomputed dropped-tail
                          # mass and inverts in one op.
                          if done[bi] == total[bi]:
                              denom(bi)
                  while k_inter < len(inter):
                      inter[k_inter]()
                      k_inter += 1

              if dbg in ("proj_only", "nodma", "p1only"):
                  with loop_ctx:
                      phase1(0)
                      for tau in range(NT):
                          if tau >= 1 and dbg != "p1only":
                              phase3(tau - 1)
                          if tau + 1 < NT:
                              phase1(tau + 1)
                      if dbg != "p1only":
                          phase3(NT - 1)
              elif defer:
                  # Uniform rounds: round tau interleaves phase3 of the
                  # PREVIOUS round ((tau-1) mod 4: round 0 drains the prior
                  # iteration's round 3) and phase1 of the NEXT round
                  # ((tau+1) mod 4: round 3 prefetches the next iteration's
                  # round 0 into the other v buffer).  The first iteration's
                  # deferred phase3(3) runs on initialized garbage and is
                  # overwritten; the final phase3(3) runs after the loop.
                  phase1(0, 0)  # prologue, outside the hw loop
                  nbody = (unroll if reps > 1
                           else int(os.environ.get("BASSBODIES", "1")))
                  with loop_ctx:
                      for body in range(nbody):
                          vb = body % 2
                          for tau in range(NT):
                              units = phase3_units((tau - 1) % NT)
                              nvb = ((vb + 1) % 2
                                     if tau + 1 == NT else vb)
                              units += phase1_units((tau + 1) % NT, nvb)
                              nh = (len(units) + 1) // 2
                              attn_pair(tau, 0, 3, units[:nh], vb)
                              attn_pair(tau, 1, 2, units[nh:], vb)
                  phase3(NT - 1)  # epilogue: the last iteration's round 3
              else:
                  with loop_ctx:
                      phase1(0, 0)
                      for tau in range(NT):
                          units = []
                          if tau >= 1:
                              units += phase3_units(tau - 1)
                          if tau + 1 < NT:
                              units += phase1_units(tau + 1, 0)
                          nh = (len(units) + 1) // 2
                          attn_pair(tau, 0, 3, units[:nh], 0)
                          attn_pair(tau, 1, 2, units[nh:], 0)
                      phase3(NT - 1)

    nc.compile()
    _PROG[cache_key] = nc
    return nc


# --------------------------------------------------------------------------
# Host-side input preparation
# --------------------------------------------------------------------------
def _split2(v):
    v = v.astype(np.float32)
    p1 = v.astype(np.float16).astype(np.float32)
    p2 = (v - p1).astype(np.float16)
    return p1.astype(np.float16), p2


def _computed_mask_for_slot(Dm, weoff_s, wed_s):
    """[T, T] bool over (i, j): True where the pair is computed on-device."""
    keep = np.zeros((T, T), dtype=bool)
    for tau in range(NT):
        i0 = 512 * tau
        for jt in range(4 * tau):
            we = weoff_s[(tau, jt)]
            if we > 0:
                keep[i0:i0 + we, 128 * jt:128 * jt + 128] = True
        for dd in range(4):
            we_d = wed_s[dd]
            j0 = i0 + 128 * dd
            c = np.arange(we_d)[:, None]
            p = np.arange(128)[None, :]
            m = (c - p >= 0) & (c - p <= Dm)
            keep[i0 + IL[dd]:i0 + IL[dd] + we_d, j0:j0 + 128] = m
    return keep


def _host_prep(x, w_qkv, w_out, alibi_slopes):
    x = np.asarray(x, np.float32)
    w_qkv = np.asarray(w_qkv, np.float32)
    w_out = np.asarray(w_out, np.float32)
    slopes = np.asarray(alibi_slopes, np.float32)
    quartets, Dm, weoff, wed, _ = _plan(slopes)

    iarr = np.arange(T, dtype=np.float32)
    # per-slot 0.5-valued tri masks: 0.5 iff 0 <= c - p <= Dm[s]
    p = np.arange(128)[:, None]
    c = np.arange(512)[None, :]
    trimask = np.zeros((128, 4, 512), np.float16)
    for s in range(4):
        trimask[:, s, :] = (((c - p) >= 0) & ((c - p) <= Dm[s])).astype(
            np.float16) * np.float16(0.5)

    # dropped-pair masks + per-(slot-geometry) distance weights are shared
    # across cores; the tail itself depends on the head's slope.
    rel = np.arange(T)[:, None] - np.arange(T)[None, :]
    causal_valid = rel >= 0
    dropped_s = []
    for s in range(4):
        weoff_s = {(tau, jt): weoff[(s, tau, jt)]
                   for tau in range(NT) for jt in range(4 * tau)}
        keep = _computed_mask_for_slot(Dm[s], weoff_s, wed[s])
        dropped_s.append((~keep) & causal_valid)

    def tail_for(s, slope):
        d = np.abs(rel).astype(np.float32) * np.float32(slope)
        num_a = 0.5 * (1.0 - d / np.sqrt(1.0 + d * d))
        return (num_a * dropped_s[s]).sum(axis=1).astype(np.float32)  # [T]

    in_maps = []
    for cc in range(NCORES):
        b = cc // 4
        g = cc % 4
        heads = [quartets[s][g] for s in range(HPC)]

        # pre-swizzled to the SBUF tile layout: [tau, half, p, k, t]
        xTf = np.ascontiguousarray(x[b].T).astype(np.float16)
        xT = np.ascontiguousarray(
            xTf.reshape(2, 4, 128, 4, 512).transpose(3, 0, 2, 1, 4))

        q_rows = np.concatenate(
            [w_qkv[64 * h:64 * h + 64] for h in heads], axis=0) * SCALE
        k_rows = np.concatenate(
            [w_qkv[C + 64 * h:C + 64 * h + 64] for h in heads], axis=0)
        qk_rows = np.concatenate([q_rows, k_rows], axis=0)  # [512, 1024]
        wqk = np.ascontiguousarray(
            qk_rows.T.reshape(8, 128, 512).transpose(1, 0, 2)).astype(np.float16)

        v_rows = np.concatenate(
            [w_qkv[2 * C + 64 * h:2 * C + 64 * h + 64] for h in heads], axis=0)
        wv = np.ascontiguousarray(
            v_rows.T.reshape(8, 128, 256).transpose(1, 0, 2)).astype(np.float16)

        Wg = np.concatenate(
            [w_out[:, 64 * h:64 * h + 64] for h in heads], axis=1)  # [1024,256]
        wo = np.ascontiguousarray(
            Wg.T.reshape(2, 128, 1024).transpose(1, 0, 2)).astype(np.float16)

        qext = np.zeros((4, HPC, T), np.float16)
        kext = np.zeros((4, HPC, T), np.float16)
        tail = np.zeros((1, HPC, T), np.float16)
        for j, h in enumerate(heads):
            sl = float(slopes[h])
            ihi, ilo = _split2(-iarr * sl)
            jhi, jlo = _split2(iarr * sl)
            qext[0, j] = ihi
            qext[1, j] = ilo
            qext[2, j] = 1.0
            qext[3, j] = 1.0
            kext[0, j] = 1.0
            kext[1, j] = 1.0
            kext[2, j] = jhi
            kext[3, j] = jlo
            tail[0, j, :] = tail_for(j, sl).astype(np.float16)

        in_maps.append({
            "xT": xT, "wqk": wqk, "wv": wv, "wo": wo,
            "qext": qext, "kext": kext, "trimask": trimask, "tail": tail,
        })
    return in_maps


def _assemble(partials):
    out = np.zeros((B, T, C), np.float32)
    for c in range(NCORES):
        out[c // 4] += partials[c]
    return out.astype(np.float32)


def kernel(x, w_qkv, w_out, alibi_slopes):
    nc = _build_program(slopes=alibi_slopes)
    in_maps = _host_prep(x, w_qkv, w_out, alibi_slopes)
    res = run_bass_kernel_spmd(nc, in_maps, core_ids=list(range(NCORES)))
    return _assemble([r["out_p"] for r in res.results])


# revision 58
# speedup vs baseline: 2.0550x; 1.0054x over previous
"""Trainium2 Bass kernel for nn_AlgebraicAttention (8-core SPMD).

Sharding: core c -> batch b = c//4, head quartet column g = c%4.  Heads are
sorted by ALiBi reach d_h = 17/slope_h (descending) and grouped into four
rank-quartets; program head-slot s on core g runs head quartets[s][g].  Every
core executes the identical program with identical tile geometry (required:
one SPMD program for all 8 cores); per-core data (weights, tail constants)
carries the head differences.  Each core computes its 4 heads' attention and
a partial out-projection; the host sums the 8 partials.

Math notes:
  - scores^T layout [j (keys, partitions), i (queries, free)].
  - ALiBi (j-i)*slope folded into the QK^T contraction via 4 extra f16 rows
    (hi/lo splits of -i*slope and j*slope).
  - geometric cut: per slot, only (i,j) pairs with dist = i-j <= Dm[slot]
    (Dm = ceil(17/min_slope_of_quartet)) are computed.  Off-diag tiles
    narrow to we columns (64-rounded); fully-far tiles drop entirely; diag
    windows narrow to wed and the per-slot 0.5-valued tri mask also zeroes
    pairs with dist > Dm.  The dropped pairs' contribution to the softmax
    DENOMINATOR is systematic (all-positive); it is precomputed on the host
    as tail(h, i) = sum_dropped num(alibi) and injected by the per-block
    o_ps opener matmul (1-row: [1s|0s] x tail row).  The dropped numerator
    contribution is a random-sign sum of O(1e-3) weights -> negligible.
  - rational softmax numerator num = 0.5*(1 + x/sqrt(1+x^2)) computed as
    sin(arctan(x)): ScalarE arctan, then per-tile either a fused deg-5
    sin-poly DVE op (SINF with tri mask folded / SINA with +0.5 affine) or
    a ScalarE Sin pass + cheap DVE finisher, chosen by a build-time greedy
    min-max over modeled engine loads.
  - the denominator is a ones-column in the P@V matmul, inverted with
    reciprocal_approx_fast; phase1/phase3 projection matmul groups are
    interleaved between attention tiles (deferred across round and
    iteration boundaries, v double-buffered) so no engine starves; the
    For_i all-engine barrier is amortized by a 4-body unroll.
"""

import numpy as np

import concourse.bass as bass
import concourse.mybir as mybir
from concourse import bacc
from concourse.tile import TileContext
from concourse.bass_utils import run_bass_kernel_spmd

# --------------------------------------------------------------------------
# Custom DVE op: out = approx 1/(Src0 + Src1) (1-Newton, ~0.17% max rel err)
# --------------------------------------------------------------------------
import concourse.dve_ops as dve_ops
from concourse.dve_ops import DveOp
from concourse.dve_spec import (
    AluOp, Bin, C0, C1, C2, C3, Spec, Src0, Src1, _spill_c3_to_src1, lower, sq,
)
from concourse.dve_uop import DveOpSpec

RC0 = -0.23548383
RC1 = 2.00161239
RC2 = 1.00011986
AB0 = RC0 * float(np.sqrt(RC2))
AB1 = RC1 * float(np.sqrt(RC2))

# deg-5 odd minimax sin on arctan range (max num abs err ~3e-5)
S5C0 = 0.9997329
S5C1 = -0.16575311
S5C2 = 0.00754758


def _notf(a):
    return (~np.asarray(a, np.float32).view(np.int32)).view(np.float32)


def _ref_recipt(in0, in1, c0, c1, c2):
    s = np.asarray(in0, np.float32) + np.asarray(in1, np.float32)
    y0 = _notf(s) * np.float32(c0)
    return (y0 * (np.float32(c1) - s * y0)).astype(np.float32)


def _spec_recipt():
    s = Bin(AluOp.ADD, Src0, Src1)
    n = Bin(AluOp.BITWISE_NOT, s, s)
    y0 = n * C0
    y1 = y0 * (C1 - s * y0)
    return Spec(body=y1, reference=_ref_recipt)


def _ref_sinf(in0, in1, c0, c1, c2):
    # full masked num: tri * (1 + theta*P(theta^2)); tri carries the 0.5
    th = np.asarray(in0, np.float32)
    m = np.asarray(in1, np.float32)
    u = th * th
    p = np.float32(c0) + u * (np.float32(c1) + u * np.float32(c2))
    return (m * (1.0 + th * p)).astype(np.float32)


def _spec_sinf():
    from concourse.dve_spec import One
    u = sq(Src0)
    p = C0 + u * (C1 + u * C2)
    s = Src0 * p
    return Spec(body=Src1 * Bin(AluOp.ADD, One, s), reference=_ref_sinf)


def _ref_trim(in0, in1, c0, c1, c2):
    # diag finisher for the ScalarE-sin path: tri * (1 + sin)
    return (np.asarray(in1, np.float32)
            * (1.0 + np.asarray(in0, np.float32))).astype(np.float32)


def _spec_trim():
    from concourse.dve_spec import One
    return Spec(body=Src1 * Bin(AluOp.ADD, One, Src0), reference=_ref_trim)


def _ref_sina(in0, in1, c0, c1, c2):
    # 0.5 + theta*P(theta^2) with 0.5-scaled coeffs; in1 = [P,1] 0.5 const
    th = np.asarray(in0, np.float32)
    u = th * th
    p = np.float32(c0) + u * (np.float32(c1) + u * np.float32(c2))
    return (np.asarray(in1, np.float32) + th * p).astype(np.float32)


def _spec_sina():
    u = sq(Src0)
    p = C0 + u * (C1 + u * C2)
    return Spec(body=_spill_c3_to_src1(C3 + Src0 * p), reference=_ref_sina)


def _register(name, spec, subdim=False):
    for op in dve_ops.OPS:
        if op.name == name:
            return op
    opcode = dve_ops._CUSTOM_DVE_ROW_BASE + len(dve_ops.OPS)
    assert opcode < 0x20
    rd1_en = dve_ops.has_src1(spec)
    shas = {}
    for ver in ("v3", "v4"):
        try:
            uops = lower(spec, ver=ver)
            shas[ver] = DveOpSpec(name=name, opcode=opcode, uops=uops,
                                  rd1_en=rd1_en).sha(ver)
        except Exception:
            pass
    op = DveOp(name, spec, subdim, uops_sha=shas)
    dve_ops.OPS.append(op)
    dve_ops._SUB_OPCODE_FOR_NAME[name] = opcode
    dve_ops.CUSTOM_DVE_SPECS[name] = spec
    return op


RECIPT_ANT = _register("RECIPT_ANT", _spec_recipt())
SINF_ANT = _register("SINF_ANT", _spec_sinf())
SINA_ANT = _register("SINA_ANT", _spec_sina())
TRIM_ANT = _register("TRIM_ANT", _spec_trim())

# diag-tile geometry: for dd = jt-4*tau in 0..3 the i-window of the
# [128 j, 512 i] o_ps block is [IL[dd], IL[dd]+WD[dd]); within it
# dist = c - p (window col c, partition p).
IL = [0, 128, 256, 384]
WD = [512, 384, 256, 128]

# --------------------------------------------------------------------------
# Problem constants
# --------------------------------------------------------------------------
B, T, C, H, D = 2, 2048, 1024, 16, 64
NCORES = 8
HPC = 4                 # heads per core
SCALE = 1.0 / 8.0       # 1/sqrt(D)
DEXT = D + 4            # q/k + [islope_hi, islope_lo, 1, 1] / [1, 1, jhi, jlo]
NT = T // 512           # 4 i-chunks of 512
NJT = T // 128          # 16 j-tiles of 128
DFAR = 11.0             # |alibi| cut distance: beyond it num < ~2e-3 and
                        # the host tail constant covers the dropped mass

F32 = mybir.dt.float32
F16 = mybir.dt.float16
AF = mybir.ActivationFunctionType

_PROG = {}


def _ceil64(x):
    return int(np.ceil(x / 64.0)) * 64


def _plan(slopes=None):
    """Head->slot assignment and per-slot computed-width tables."""
    import os
    dfar = float(os.environ.get("BASSDFAR", str(DFAR)))
    if slopes is None:
        start = 2.0 ** (-8.0 / H)
        slopes = np.asarray([start ** (i + 1) for i in range(H)], np.float32)
    slopes = np.asarray(slopes, np.float32)
    d = dfar / np.maximum(np.abs(slopes), 1e-12)
    order = np.argsort(-d, kind="stable")
    quartets = [order[4 * s:4 * s + 4].tolist() for s in range(4)]
    Dm = [int(np.ceil(max(float(d[h]) for h in quartets[s])))
          for s in range(4)]
    weoff = {}
    for s in range(4):
        for tau in range(NT):
            for jt in range(4 * tau):
                we = 128 * jt + 128 + Dm[s] - 512 * tau
                weoff[(s, tau, jt)] = min(max(_ceil64(we), 0), 512)
    wed = [[min(WD[dd], _ceil64(128 + Dm[s])) for dd in range(4)]
           for s in range(4)]
    key = tuple(Dm)
    return quartets, Dm, weoff, wed, key


# --------------------------------------------------------------------------
# Device program (identical on all 8 cores)
# --------------------------------------------------------------------------
def _build_program(reps=1, slopes=None):
    import os
    dbg = os.environ.get("BASSDBG", "")
    nopool = os.environ.get("BASSNOPOOL", "1") == "1"
    nofuse = os.environ.get("BASSNOFUSE", "") == "1"
    sccopy = os.environ.get("BASSSCCOPY", "1") == "1"
    defer = os.environ.get("BASSDEFER", "1") == "1"
    unroll = int(os.environ.get("BASSUNROLL", "4"))
    psum_cfg = (os.environ.get("BASSACC", "2"), os.environ.get("BASSPS", "4"),
                os.environ.get("BASSPSO", "2"), os.environ.get("BASSEW", "6"))
    fbias = float(os.environ.get("BASSFB", "1.0"))
    _, Dm, weoff, wed, key = _plan(slopes)
    cache_key = (reps, key, dbg, nopool, nofuse, sccopy, defer, unroll,
                 psum_cfg, fbias)
    if cache_key in _PROG:
        return _PROG[cache_key]

    nc = bacc.Bacc("TRN2", target_bir_lowering=False, debug=False,
                   num_devices=NCORES)

    d_xT = nc.dram_tensor("xT", [NT, 2, 128, 4, 512], F16,
                          kind="ExternalInput")
    d_wqk = nc.dram_tensor("wqk", [128, 8, 512], F16, kind="ExternalInput")
    d_wv = nc.dram_tensor("wv", [128, 8, 256], F16, kind="ExternalInput")
    d_wo = nc.dram_tensor("wo", [128, 2, 1024], F16, kind="ExternalInput")
    d_qext = nc.dram_tensor("qext", [4, 4, T], F16, kind="ExternalInput")
    d_kext = nc.dram_tensor("kext", [4, 4, T], F16, kind="ExternalInput")
    d_tri = nc.dram_tensor("trimask", [128, 4, 512], F16,
                           kind="ExternalInput")
    d_tail = nc.dram_tensor("tail", [1, 4, T], F16, kind="ExternalInput")
    d_out = nc.dram_tensor("out_p", [T, C], F16, kind="ExternalOutput")

    with TileContext(nc) as tc:
        with (
            tc.tile_pool(name="const", bufs=1) as cpool,
            tc.tile_pool(name="ew", bufs=int(os.environ.get("BASSEW", "6"))
                         ) as ew,
            tc.tile_pool(name="osb", bufs=3) as osb,
            tc.tile_pool(name="acc", bufs=int(os.environ.get("BASSACC", "2")),
                         space="PSUM") as accp,
            tc.tile_pool(name="ps", bufs=int(os.environ.get("BASSPS", "4")),
                         space="PSUM") as psp,
            tc.tile_pool(name="pso", bufs=int(os.environ.get("BASSPSO", "2")),
                         space="PSUM") as psop,
        ):
            # ---------------- persistent tensors ----------------
            wqk_sb = cpool.tile([128, 8, 512], F16, tag="wqk")
            wv_sb = cpool.tile([128, 8, 256], F16, tag="wv")
            wo_sb = cpool.tile([128, 2, 1024], F16, tag="wo")
            q_all = cpool.tile([128, HPC, T], F16, tag="q_all")
            k_all = cpool.tile([128, HPC, T], F16, tag="k_all")
            # v double-buffered by iteration parity: lets the next
            # iteration's V projection start while this iteration's last
            # pairs still read the current buffer.
            v_sb = cpool.tile([128, 2, NJT, HPC * 128], F16, tag="v_sb")
            o_all = cpool.tile([128, 2, T], F16, tag="o_all")
            tri = cpool.tile([128, 4, 512], F16, tag="tri")
            tail_sb = cpool.tile([1, 4, T], F16, tag="tail")
            xsb = cpool.tile([128, NT, 2, 4, 512], F16, tag="xsb")

            nc.sync.dma_start(wqk_sb[:], d_wqk[:])
            nc.sync.dma_start(wv_sb[:], d_wv[:])
            nc.sync.dma_start(wo_sb[:], d_wo[:])
            nc.sync.dma_start(tri[:], d_tri[:])
            nc.sync.dma_start(tail_sb[:], d_tail[:])
            for tau in range(NT):
                for half in range(2):
                    nc.sync.dma_start(xsb[:, tau, half], d_xT[tau, half])

            # constants: hoisted out of the timing rep-loop (idempotent).
            # ext rows:   even slots at rows [64:68) (matmul reads [0:68)),
            # odd slots at rows [60:64) with zeros in [0:60) (matmul reads
            # [0:128) — ldweights requires partition base 0 for >32 rows).
            for h in range(HPC):
                if h % 2 == 1:
                    nc.vector.memset(q_all[0:64, h, :], 0.0)
                    nc.vector.memset(k_all[0:64, h, :], 0.0)
                base = 64 if h % 2 == 0 else 60
                nc.sync.dma_start(q_all[base:base + 4, h, :], d_qext[:, h, :])
                nc.sync.dma_start(k_all[base:base + 4, h, :], d_kext[:, h, :])

            half = cpool.tile([128, 1], F32, tag="half")
            nc.vector.memset(half[:], 0.5)
            # [1,128] selector for the o_ps opener: ones over the
            # denominator partitions, zeros over the value partitions.
            dcol = cpool.tile([1, 128], F16, tag="dcol")
            nc.vector.memset(dcol[:, 0:64], 1.0)
            nc.vector.memset(dcol[:, 64:128], 0.0)
            if dbg == "noew":
                num_const = cpool.tile([128, 512], F16, tag="numc")
                nc.vector.memset(num_const[:], 0.001)
            # o_all is read by the deferred phase3(3) before the first
            # iteration writes it — initialize to keep the garbage finite.
            nc.vector.memset(o_all[:], 0.001)
            # ones columns of V_ext in cols 0:64 of each head's group, so
            # the P@V denominator lands at o_ps partitions [0:64) (custom DVE
            # ops require partition base 0 on their input).
            v4 = v_sb[:].rearrange("p b t (h e) -> p b t h e", e=128)
            nc.gpsimd.memset(v4[:, :, :, :, 0:64], 1.0)

            import contextlib
            # `unroll` bodies per hw-loop iteration (v ping-pong):
            # amortizes the For_i all-engine barrier; effective iteration
            # count is unroll*ceil(reps/unroll) for reps > 1.
            loop_ctx = (tc.For_i(0, (reps + unroll - 1) // unroll, 1)
                        if reps > 1 else contextlib.nullcontext())
            if True:
              # Issue order per round tau:
              #   pairA(tau) -> phase3(tau-1) -> pairB(tau) -> phase1(tau+1)
              # The PE-only projection segments are sandwiched between
              # attention pairs, so the elementwise engines drain their
              # attention backlog while the PE runs projections, instead of
              # idling per round (phase1(0) is the prologue, phase3(NT-1)
              # the epilogue).
              LA = 5  # QK lookahead within a pair (psp ring bounds it too)

              # build-time per-engine load model (ns) for assigning each
              # tile's sin+finisher to ScalarE/DVE/Pool (greedy min-max).
              LOAD = {"sc": 0.0, "dve": 0.0, "pool": 0.0}

              def p1_v_unit(tau, ttl, vb):
                  xa = xsb[:, tau, 0]
                  xb = xsb[:, tau, 1]
                  tt = 4 * tau + ttl
                  accv = accp.tile([128, 256], F32, tag="acc", name="accv")
                  for kt in range(8):
                      xt = xa if kt < 4 else xb
                      nc.tensor.matmul(
                          accv[:], xt[:, kt % 4, 128 * ttl:128 * ttl + 128],
                          wv_sb[:, kt, :],
                          start=(kt == 0), stop=(kt == 7))
                  nc.vector.tensor_copy(
                      out=v4[:, vb, tt, :, 64:128],
                      in_=accv[:].rearrange("p (h e) -> p h e", e=64))
                  LOAD["dve"] += 327

              def p1_qk_unit(tau, mt):
                  ts = slice(512 * tau, 512 * tau + 512)
                  xa = xsb[:, tau, 0]
                  xb = xsb[:, tau, 1]
                  acc = accp.tile([128, 512], F32, tag="acc", name="acc")
                  for kt in range(8):
                      xt = xa if kt < 4 else xb
                      nc.tensor.matmul(
                          acc[:], wqk_sb[:, kt, 128 * mt:128 * mt + 128],
                          xt[:, kt % 4, :],
                          start=(kt == 0), stop=(kt == 7))
                  dst = q_all if mt < 2 else k_all
                  h0 = 2 * (mt % 2)
                  for (p0, hh) in ((0, h0), (64, h0 + 1)):
                      if sccopy and LOAD["sc"] + 594 <= LOAD["dve"] + 594:
                          nc.scalar.activation(
                              dst[p0:p0 + 64, hh, ts],
                              acc[p0:p0 + 64, :], AF.Copy)
                          LOAD["sc"] += 594
                      else:
                          nc.vector.tensor_copy(
                              out=dst[p0:p0 + 64, hh, ts],
                              in_=acc[p0:p0 + 64, :])
                          LOAD["dve"] += 594

              def phase1_units(tau, vb):
                  # V first: the next round's diag/tri matmuls need it
                  return ([lambda ttl=ttl: p1_v_unit(tau, ttl, vb)
                           for ttl in range(4)]
                          + [lambda mt=mt: p1_qk_unit(tau, mt)
                             for mt in (0, 2, 1, 3)])

              def p3_unit(tau, ttl, oc):
                  tt = 4 * tau + ttl
                  acc = accp.tile([128, 512], F32, tag="acc", name="acc3")
                  for half in range(2):
                      nc.tensor.matmul(
                          acc[:],
                          o_all[:, half, 128 * tt:128 * tt + 128],
                          wo_sb[:, half, 512 * oc:512 * oc + 512],
                          start=(half == 0), stop=(half == 1))
                  ot = osb.tile([128, 512], F16, tag="ot", name="ot")
                  if sccopy and LOAD["sc"] + 594 <= LOAD["dve"] + 594:
                      nc.scalar.activation(ot[:], acc[:], AF.Copy)
                      LOAD["sc"] += 594
                  else:
                      nc.vector.tensor_copy(out=ot[:], in_=acc[:])
                      LOAD["dve"] += 594
                  if dbg != "nodma":
                      nc.sync.dma_start(
                          d_out[128 * tt:128 * tt + 128,
                                512 * oc:512 * oc + 512],
                          ot[:])

              def phase3_units(tau):
                  return [lambda ttl=ttl, oc=oc: p3_unit(tau, ttl, oc)
                          for ttl in range(4) for oc in range(2)]

              def phase1(tau, vb=0):
                  for u in phase1_units(tau, vb):
                      u()

              def phase3(tau):
                  for u in phase3_units(tau):
                      u()

              def attn_pair(tau, hA, hB, inter=(), vb=0):
                  i0 = 512 * tau
                  isl = slice(i0, i0 + 512)
                  njt = 4 * (tau + 1)
                  blocks = []
                  for h in (hA, hB):
                      blocks.append(dict(
                          h=h,
                          hb=0, hk=(DEXT if h % 2 == 0 else 128),
                          o_ps=psop.tile([128, 512], F32, tag="pso",
                                         name="o_ps")))

                  def geom(h, n):
                      dd = n - 4 * tau
                      if dd >= 0:
                          return IL[dd], wed[h][dd]
                      return 0, weoff[(h, tau, n)]

                  def qk(bi, n):
                      blk = blocks[bi]
                      il, wd = geom(blk["h"], n)
                      x_ps = psp.tile([128, 512], F32, tag="ps", name="x_ps")
                      nc.tensor.matmul(
                          x_ps[:, 0:wd],
                          k_all[blk["hb"]:blk["hb"] + blk["hk"],
                                blk["h"], 128 * n:128 * n + 128],
                          q_all[blk["hb"]:blk["hb"] + blk["hk"],
                                blk["h"], i0 + il:i0 + il + wd],
                          start=True, stop=True)
                      return x_ps

                  # wide tiles first, narrow tiles last: the round's tail is
                  # then short elementwise chains, minimizing the in-order PE
                  # bubble at the pair boundary.
                  sched = [(bi, n) for n in range(njt) for bi in (0, 1)
                           if geom(blocks[bi]["h"], n)[1] > 0]
                  sched.sort(key=lambda s: -geom(blocks[s[0]]["h"], s[1])[1])
                  total = {0: 0, 1: 0}
                  for bi, n in sched:
                      total[bi] += 1

                  tiles = {}
                  for idx in range(min(LA, len(sched))):
                      tiles[sched[idx]] = qk(*sched[idx])
                  # group opener: one 1-row matmul zero-fills each o_ps and
                  # injects the host-precomputed dropped-tail mass into the
                  # denominator rows (dcol = [1s x64 | 0s x64]).
                  for bi in (0, 1):
                      h = blocks[bi]["h"]
                      nc.tensor.matmul(
                          blocks[bi]["o_ps"][:],
                          dcol[:], tail_sb[0:1, h, isl],
                          start=True, stop=False,
                          skip_group_check=True)
                  def denom(bi):
                      # emitted as soon as the block's last PV lands: frees
                      # the psop bank early for the next pair.  The tail
                      # mass is already in the denominator via the opener.
                      h = blocks[bi]["h"]
                      o_ps = blocks[bi]["o_ps"]
                      rsb = ew.tile([64, 512], F32, tag="rsb", name="rsb")
                      nc.vector.reciprocal_approx_fast(out=rsb[:],
                                                       in_=o_ps[0:64, :])
                      nc.vector.tensor_mul(
                          out=o_all[64 * (h % 2):64 * (h % 2) + 64,
                                    h // 2, isl],
                          in0=o_ps[64:128, :], in1=rsb[:])
                      LOAD["dve"] += 2 * 593

                  done = {0: 0, 1: 0}
                  k_inter = 0
                  for idx, (bi, jt) in enumerate(sched):
                      # pace the interleaved PE-only projection units so the
                      # elementwise engines keep receiving fresh scores
                      # instead of starving during contiguous projection
                      # bursts.
                      want = (idx * len(inter)) // max(len(sched) - 1, 1)
                      while k_inter < want:
                          inter[k_inter]()
                          k_inter += 1
                      if idx + LA < len(sched):
                          tiles[sched[idx + LA]] = qk(*sched[idx + LA])
                      x_ps = tiles.pop((bi, jt))
                      blk = blocks[bi]
                      h = blk["h"]
                      dd = jt - 4 * tau  # >= 0 on diagonal block
                      il, wd = geom(h, jt)
                      if dbg == "noew":
                          num = num_const
                      else:
                          num = ew.tile([128, 512], F16, tag="num",
                                        name="num")
                          # x/sqrt(1+x^2) = sin(arctan(x)): ScalarE arctan,
                          # then either a ScalarE Sin pass + DVE/Pool f16
                          # finisher (affine / tri-mask mul), or a single
                          # fused deg-5 sin-poly DVE op with the finisher
                          # folded in.  Greedy min-max over the modeled
                          # engine loads picks per tile.  x_ps is freed
                          # right after the arctan pass.
                          at = ew.tile([128, 512], F32, tag="at", name="at")
                          nc.scalar.activation(at[:, 0:wd], x_ps[:, 0:wd],
                                               AF.Arctan)
                          LOAD["sc"] += wd * 1.043 + 60
                          c_sin = wd * 1.043 + 60
                          c_fin_dve = (wd * 0.52 + 60) if dd >= 0 else (
                              wd * 0.30 + 60)
                          c_fin_pool = wd * 2.48 + 95
                          c_fused = (wd * 1.043 * fbias + 125)
                          mA = max(LOAD["sc"] + c_sin,
                                   LOAD["dve"] + c_fin_dve, LOAD["pool"])
                          mB = max(LOAD["sc"] + c_sin, LOAD["dve"],
                                   LOAD["pool"] + c_fin_pool)
                          mC = max(LOAD["sc"], LOAD["dve"] + c_fused,
                                   LOAD["pool"])
                          if nopool:
                              mB = float("inf")
                          if nofuse:
                              mC = float("inf")
                          if mC <= mA and mC <= mB:
                              # fused DVE sin (full masked num / +0.5 affine)
                              LOAD["dve"] += c_fused
                              if dd >= 0:
                                  nc.vector._custom_dve(
                                      SINF_ANT, out=num[:, 0:wd],
                                      in0=at[:, 0:wd], in1=tri[:, h, 0:wd],
                                      s0=S5C0, s1=S5C1, imm2=S5C2)
                              else:
                                  nc.vector._custom_dve(
                                      SINA_ANT, out=num[:, 0:wd],
                                      in0=at[:, 0:wd], in1=half[:],
                                      s0=0.5 * S5C0, s1=0.5 * S5C1,
                                      imm2=0.5 * S5C2)
                          else:
                              un = ew.tile([128, 512], F16, tag="un",
                                           name="un")
                              nc.scalar.activation(un[:, 0:wd], at[:, 0:wd],
                                                   AF.Sin)
                              LOAD["sc"] += c_sin
                              LOAD["dve"] += c_fin_dve
                              if dd < 0:
                                  nc.vector.tensor_scalar(
                                      out=num[:, 0:wd], in0=un[:, 0:wd],
                                      scalar1=0.5, scalar2=0.5,
                                      op0=mybir.AluOpType.mult,
                                      op1=mybir.AluOpType.add)
                              else:
                                  nc.vector._custom_dve(
                                      TRIM_ANT, out=num[:, 0:wd],
                                      in0=un[:, 0:wd],
                                      in1=tri[:, h, 0:wd])
                      done[bi] += 1
                      if dbg != "nopv":
                          nc.tensor.matmul(
                              blk["o_ps"][:, il:il + wd],
                              v_sb[:, vb, jt, 128 * h:128 * h + 128],
                              num[:, 0:wd],
                              start=False, stop=(done[bi] == total[bi]),
                              skip_group_check=True)
                          # denominators (rows 0:64, replicated by the ones
                          # columns; custom-DVE requires partition base 0).
                          # RECIPT adds the host-precomputed dropped-tail
                          # mass and inverts in one op.
                          if done[bi] == total[bi]:
                              denom(bi)
                  while k_inter < len(inter):
                      inter[k_inter]()
                      k_inter += 1

              if dbg in ("proj_only", "nodma", "p1only"):
                  with loop_ctx:
                      phase1(0)
                      for tau in range(NT):
                          if tau >= 1 and dbg != "p1only":
                              phase3(tau - 1)
                          if tau + 1 < NT:
                              phase1(tau + 1)
                      if dbg != "p1only":
                          phase3(NT - 1)
              elif defer:
                  # Uniform rounds: round tau interleaves phase3 of the
                  # PREVIOUS round ((tau-1) mod 4: round 0 drains the prior
                  # iteration's round 3) and phase1 of the NEXT round
                  # ((tau+1) mod 4: round 3 prefetches the next iteration's
                  # round 0 into the other v buffer).  The first iteration's
                  # deferred phase3(3) runs on initialized garbage and is
                  # overwritten; the final phase3(3) runs after the loop.
                  phase1(0, 0)  # prologue, outside the hw loop
                  nbody = (unroll if reps > 1
                           else int(os.environ.get("BASSBODIES", "1")))
                  with loop_ctx:
                      for body in range(nbody):
                          vb = body % 2
                          for tau in range(NT):
                              units = phase3_units((tau - 1) % NT)
                              nvb = ((vb + 1) % 2
                                     if tau + 1 == NT else vb)
                              units += phase1_units((tau + 1) % NT, nvb)
                              nh = (len(units) + 1) // 2
                              attn_pair(tau, 0, 3, units[:nh], vb)
                              attn_pair(tau, 1, 2, units[nh:], vb)
                  phase3(NT - 1)  # epilogue: the last iteration's round 3
              else:
                  with loop_ctx:
                      phase1(0, 0)
                      for tau in range(NT):
                          units = []
                          if tau >= 1:
                              units += phase3_units(tau - 1)
                          if tau + 1 < NT:
                              units += phase1_units(tau + 1, 0)
                          nh = (len(units) + 1) // 2
                          attn_pair(tau, 0, 3, units[:nh], 0)
                          attn_pair(tau, 1, 2, units[nh:], 0)
                      phase3(NT - 1)

    nc.compile()
    _PROG[cache_key] = nc
    return nc


# --------------------------------------------------------------------------
# Host-side input preparation
# --------------------------------------------------------------------------
def _split2(v):
    v = v.astype(np.float32)
    p1 = v.astype(np.float16).astype(np.float32)
    p2 = (v - p1).astype(np.float16)
    return p1.astype(np.float16), p2


def _computed_mask_for_slot(Dm, weoff_s, wed_s):
    """[T, T] bool over (i, j): True where the pair is computed on-device."""
    keep = np.zeros((T, T), dtype=bool)
    for tau in range(NT):
        i0 = 512 * tau
        for jt in range(4 * tau):
            we = weoff_s[(tau, jt)]
            if we > 0:
                keep[i0:i0 + we, 128 * jt:128 * jt + 128] = True
        for dd in range(4):
            we_d = wed_s[dd]
            j0 = i0 + 128 * dd
            c = np.arange(we_d)[:, None]
            p = np.arange(128)[None, :]
            m = (c - p >= 0) & (c - p <= Dm)
            keep[i0 + IL[dd]:i0 + IL[dd] + we_d, j0:j0 + 128] = m
    return keep


def _host_prep(x, w_qkv, w_out, alibi_slopes):
    x = np.asarray(x, np.float32)
    w_qkv = np.asarray(w_qkv, np.float32)
    w_out = np.asarray(w_out, np.float32)
    slopes = np.asarray(alibi_slopes, np.float32)
    quartets, Dm, weoff, wed, _ = _plan(slopes)

    iarr = np.arange(T, dtype=np.float32)
    # per-slot 0.5-valued tri masks: 0.5 iff 0 <= c - p <= Dm[s]
    p = np.arange(128)[:, None]
    c = np.arange(512)[None, :]
    trimask = np.zeros((128, 4, 512), np.float16)
    for s in range(4):
        trimask[:, s, :] = (((c - p) >= 0) & ((c - p) <= Dm[s])).astype(
            np.float16) * np.float16(0.5)

    # dropped-pair masks + per-(slot-geometry) distance weights are shared
    # across cores; the tail itself depends on the head's slope.
    rel = np.arange(T)[:, None] - np.arange(T)[None, :]
    causal_valid = rel >= 0
    dropped_s = []
    for s in range(4):
        weoff_s = {(tau, jt): weoff[(s, tau, jt)]
                   for tau in range(NT) for jt in range(4 * tau)}
        keep = _computed_mask_for_slot(Dm[s], weoff_s, wed[s])
        dropped_s.append((~keep) & causal_valid)

    def tail_for(s, slope):
        d = np.abs(rel).astype(np.float32) * np.float32(slope)
        num_a = 0.5 * (1.0 - d / np.sqrt(1.0 + d * d))
        return (num_a * dropped_s[s]).sum(axis=1).astype(np.float32)  # [T]

    in_maps = []
    for cc in range(NCORES):
        b = cc // 4
        g = cc % 4
        heads = [quartets[s][g] for s in range(HPC)]

        # pre-swizzled to the SBUF tile layout: [tau, half, p, k, t]
        xTf = np.ascontiguousarray(x[b].T).astype(np.float16)
        xT = np.ascontiguousarray(
            xTf.reshape(2, 4, 128, 4, 512).transpose(3, 0, 2, 1, 4))

        q_rows = np.concatenate(
            [w_qkv[64 * h:64 * h + 64] for h in heads], axis=0) * SCALE
        k_rows = np.concatenate(
            [w_qkv[C + 64 * h:C + 64 * h + 64] for h in heads], axis=0)
        qk_rows = np.concatenate([q_rows, k_rows], axis=0)  # [512, 1024]
        wqk = np.ascontiguousarray(
            qk_rows.T.reshape(8, 128, 512).transpose(1, 0, 2)).astype(np.float16)

        v_rows = np.concatenate(
            [w_qkv[2 * C + 64 * h:2 * C + 64 * h + 64] for h in heads], axis=0)
        wv = np.ascontiguousarray(
            v_rows.T.reshape(8, 128, 256).transpose(1, 0, 2)).astype(np.float16)

        Wg = np.concatenate(
            [w_out[:, 64 * h:64 * h + 64] for h in heads], axis=1)  # [1024,256]
        wo = np.ascontiguousarray(
            Wg.T.reshape(2, 128, 1024).transpose(1, 0, 2)).astype(np.float16)

        qext = np.zeros((4, HPC, T), np.float16)
        kext = np.zeros((4, HPC, T), np.float16)
        tail = np.zeros((1, HPC, T), np.float16)
        for j, h in enumerate(heads):
            sl = float(slopes[h])
            ihi, ilo = _split2(-iarr * sl)
            jhi, jlo = _split2(iarr * sl)
            qext[0, j] = ihi
            qext[1, j] = ilo
            qext[2, j] = 1.0
            qext[3, j] = 1.0
            kext[0, j] = 1.0
            kext[1, j] = 1.0
            kext[2, j] = jhi
            kext[3, j] = jlo
            tail[0, j, :] = tail_for(j, sl).astype(np.float16)

        in_maps.append({
            "xT": xT, "wqk": wqk, "wv": wv, "wo": wo,
            "qext": qext, "kext": kext, "trimask": trimask, "tail": tail,
        })
    return in_maps


def _assemble(partials):
    out = np.zeros((B, T, C), np.float32)
    for c in range(NCORES):
        out[c // 4] += partials[c]
    return out.astype(np.float32)


def kernel(x, w_qkv, w_out, alibi_slopes):
    nc = _build_program(slopes=alibi_slopes)
    in_maps = _host_prep(x, w_qkv, w_out, alibi_slopes)
    res = run_bass_kernel_spmd(nc, in_maps, core_ids=list(range(NCORES)))
    return _assemble([r["out_p"] for r in res.results])
